# revision 2
# baseline (speedup 1.0000x reference)
"""Att_RNN_GRU Trainium2 Bass kernel, latency-optimized recurrence.

Data-parallel over batch: B=128 split across 8 cores (16 each).
Layout strategy: channels-on-partitions, (batch,time)-on-free everywhere.

Chain-minimized GRU step (critical path after the psum matmuls):
  sigma(ps[0:4]) -> rn = ps_n * r -> npre = rn + gx_n -> tanh -> nzm=(z-1)*n
h is kept as the UNMATERIALIZED pair (hz, -nzm): the next step's matmuls
accumulate W*hz (via whh) and W*nzm (via negated weights whhN), so the
h materialization (hs history write, m = hz - nzm) happens off the
critical path on gpsimd.  b_hn enters psum via a constant-tile identity
matmul (inject2); the r,z,n gx enter via inject1 (identity matmul on the
precomputed gx chunk, biases baked).
Attention phase unchanged from baseline (fp16 um/softmax/context).
"""

import os

import numpy as np

import concourse.bass as bass
import concourse.mybir as mybir
from concourse import bacc
from concourse import bass_utils as _bu
from concourse.bass_utils import run_bass_kernel_spmd

_orig_run_command = _bu.run_command


def _run_command_nobs(cmd, **kw):
    cmd = [
        ("--enable-birsim=false" if c == "--enable-birsim=true" else c) for c in cmd
    ]
    return _orig_run_command(cmd, **kw)


_bu.run_command = _run_command_nobs
from concourse.tile import TileContext

B, T, I, H, A = 128, 1024, 128, 256, 40
NCORES = 8
BL = B // NCORES  # 16 batch rows per core
KH = H // 128  # 2 hidden k-chunks
M3 = 3 * H // 128  # 6 gate m-chunks (r: 0,1  z: 2,3  n: 4,5)
CH = 64  # timesteps per gx chunk

f32 = mybir.dt.float32
f16 = mybir.dt.float16
WDT_F16 = os.environ.get("RNN_WDT", "f16") == "f16"
TWOSUM = os.environ.get("RNN_TWOSUM", "1") == "1"
DEBUG_HS = os.environ.get("RNN_DEBUG_HS", "0") == "1"
USE_TTR = os.environ.get("RNN_TTR", "0") == "1"
wdt = f16 if WDT_F16 else f32

AF = mybir.ActivationFunctionType
ALU = mybir.AluOpType
AX = mybir.AxisListType


def build_program(T_=None, CH_=None):
    T_ = T_ or int(os.environ.get("RNN_T", T))
    CH_ = CH_ or int(os.environ.get("RNN_CH", CH))
    nchunk = T_ // CH_
    assert T_ % CH_ == 0

    nc = bacc.Bacc(
        "TRN2", target_bir_lowering=False, debug=False, num_devices=NCORES
    )
    x_ext = nc.declare_dram_parameter("x", [BL, T_, I], f32, isOutput=False)
    whhT = nc.declare_dram_parameter("W_hhT", [H, 3 * H], wdt, isOutput=False)
    whhTN = nc.declare_dram_parameter("W_hhTN", [H, 3 * H], wdt, isOutput=False)
    bhnp = nc.declare_dram_parameter("bhn_pack", [128, 2 * BL], wdt, isOutput=False)
    wihT = nc.declare_dram_parameter("W_ihT", [I, 3 * H], wdt, isOutput=False)
    biasp = nc.declare_dram_parameter("bias_pack", [128, 8], f32, isOutput=False)
    ident = nc.declare_dram_parameter("identity", [128, 128], f32, isOutput=False)
    wvT = nc.declare_dram_parameter("wv_WT", [H, A], f16, isOutput=False)
    wvb = nc.declare_dram_parameter("wv_b", [A, 1], f32, isOutput=False)
    wud = nc.declare_dram_parameter("wu_delta", [A, BL * BL], f16, isOutput=False)
    sel = nc.declare_dram_parameter("bcast_sel", [BL, BL * 128], f16, isOutput=False)
    h2o = nc.declare_dram_parameter("h2o_pack", [128, KH], f32, isOutput=False)
    h2ob = nc.declare_dram_parameter("h2o_b", [1, 1], f32, isOutput=False)
    out_ext = nc.declare_dram_parameter("out", [BL, 1], f32, isOutput=True)
    hs_dbg = (
        nc.declare_dram_parameter("hs_dbg", [128, KH * BL * T_], f16, isOutput=True)
        if DEBUG_HS
        else None
    )
    rz_dbg = (
        nc.declare_dram_parameter("rz_dbg", [128, 4 * BL], f32, isOutput=True)
        if DEBUG_HS
        else None
    )

    with TileContext(nc) as tc:
        with (
            tc.tile_pool(name="consts", bufs=1) as cpool,
            tc.tile_pool(name="hspool", bufs=1) as hspool,
        ):
            # ---------------- constants ----------------
            whh_sb = cpool.tile([128, KH, M3, 128], wdt)
            for k in range(KH):
                nc.sync.dma_start(
                    out=whh_sb[:, k],
                    in_=whhT[k * 128 : (k + 1) * 128, :].rearrange(
                        "p (m c) -> p m c", m=M3
                    ),
                )
            whhN_sb = cpool.tile([128, KH, M3, 128], wdt)
            for k in range(KH):
                nc.sync.dma_start(
                    out=whhN_sb[:, k],
                    in_=whhTN[k * 128 : (k + 1) * 128, :].rearrange(
                        "p (m c) -> p m c", m=M3
                    ),
                )
            bhn_sb = cpool.tile([128, KH, BL], wdt)
            nc.sync.dma_start(
                out=bhn_sb, in_=bhnp[:, :].rearrange("p (k b) -> p k b", k=KH)
            )
            wih_sb = cpool.tile([128, M3, 128], wdt)
            nc.sync.dma_start(
                out=wih_sb, in_=wihT[:, :].rearrange("p (m c) -> p m c", m=M3)
            )
            bias_sb = cpool.tile([128, 8], f32)
            nc.sync.dma_start(out=bias_sb, in_=biasp[:, :])
            id_sb = cpool.tile([128, 128], f32)
            nc.sync.dma_start(out=id_sb, in_=ident[:, :])
            idw_sb = cpool.tile([128, 128], wdt)
            if WDT_F16:
                nc.vector.tensor_copy(out=idw_sb, in_=id_sb)
            else:
                idw_sb = id_sb
            wv_sb = cpool.tile([128, KH, A], f16)
            for k in range(KH):
                nc.sync.dma_start(out=wv_sb[:, k], in_=wvT[k * 128 : (k + 1) * 128, :])
            wvb_sb = cpool.tile([A, 1], f32)
            nc.sync.dma_start(out=wvb_sb, in_=wvb[:, :])
            wud_sb = cpool.tile([A, BL, BL], f16)
            nc.sync.dma_start(out=wud_sb, in_=wud[:, :].rearrange("a (b c) -> a b c", b=BL))
            sel_sb = cpool.tile([BL, BL, 128], f16)
            nc.sync.dma_start(out=sel_sb, in_=sel[:, :].rearrange("a (b c) -> a b c", b=BL))
            h2o_sb = cpool.tile([128, KH], f32)
            nc.sync.dma_start(out=h2o_sb, in_=h2o[:, :])
            h2ob_sb = cpool.tile([1, 1], f32)
            nc.sync.dma_start(out=h2ob_sb, in_=h2ob[:, :])

            # hidden-state history, fp16: [128, k, b, t]
            hs_sb = hspool.tile([128, KH, BL, T_], f16)

            # ---------------- recurrence ----------------
            with (
                tc.tile_pool(name="xio", bufs=2) as xpool,
                tc.tile_pool(name="gxp", bufs=2) as gxpool,
                tc.tile_pool(name="state", bufs=6) as hpool,
                tc.tile_pool(name="scr", bufs=4) as spool,
                tc.tile_pool(name="psgr", bufs=2, space="PSUM") as psgr,
                tc.tile_pool(name="psgn", bufs=2, space="PSUM") as psgn,
                tc.tile_pool(name="pst", bufs=2, space="PSUM") as pstp,
                tc.tile_pool(name="psx", bufs=2, space="PSUM") as psxp,
            ):
                hz_prev = None
                nzm_prev = None

                for c in range(nchunk):
                    t0 = c * CH_
                    npair = BL // 2
                    ncol = BL * CH_  # gx cols this chunk
                    # ---- load x chunk, pair-major rows: [(bo t), i]
                    xin = xpool.tile([128, npair * 128], f32, tag="xin")
                    xiv = xin.rearrange("p (q c) -> p q c", q=npair)
                    for p in range(npair):
                        for bo in range(2):
                            nc.sync.dma_start(
                                out=xiv[bo * CH_ : (bo + 1) * CH_, p],
                                in_=x_ext[2 * p + bo, t0 : t0 + CH_, :],
                            )
                    # ---- transpose -> xT [i, (pair, bo, tc)]
                    xT = xpool.tile([128, ncol], wdt, tag="xT")
                    for p in range(npair):
                        ps_t = pstp.tile([128, 128], f32, tag="ps_t")
                        nc.tensor.transpose(ps_t, xiv[:, p], id_sb)
                        nc.scalar.copy(out=xT[:, p * 128 : (p + 1) * 128], in_=ps_t)
                    # ---- gx matmuls: [128, m, pair, bo, tc] with bias baked
                    # slots 0-3: r,z (bias = b_ih+b_hh); slots 4,5: n (bias = b_ih)
                    gx = gxpool.tile([128, M3, npair, 2, CH_], wdt, tag="gx")
                    gxf = gx.rearrange("p m q b t -> p m (q b t)")
                    nh = ncol // 512 if ncol >= 512 else 1
                    nsz = min(512, ncol)
                    for m in range(M3):
                        for j in range(nh):
                            ps_gx = psxp.tile([128, nsz], f32, tag="ps_gx")
                            nc.tensor.matmul(
                                ps_gx,
                                wih_sb[:, m],
                                xT[:, j * nsz : (j + 1) * nsz],
                                start=True,
                                stop=True,
                            )
                            if (m + j) % 2 == 0:
                                nc.vector.tensor_scalar_add(
                                    gxf[:, m, j * nsz : (j + 1) * nsz],
                                    ps_gx,
                                    bias_sb[:, m : m + 1],
                                )
                            else:
                                nc.scalar.activation(
                                    gxf[:, m, j * nsz : (j + 1) * nsz],
                                    ps_gx,
                                    AF.Identity,
                                    bias=bias_sb[:, m : m + 1],
                                )
                    # ---- CH_ recurrence steps
                    for tcx in range(CH_):
                        t = t0 + tcx
                        first = t == 0
                        # separate psum tiles (= separate banks/semaphores):
                        # sigma waits only on the rz writers, not the n-gate's
                        ps_rz = psgr.tile([128, 4, BL], f32, tag="ps_rz")
                        ps_n = psgn.tile([128, KH, BL], f32, tag="ps_n")
                        nc.tensor.matmul(
                            ps_rz,
                            idw_sb,
                            gx[:, 0:4, :, :, tcx],
                            start=True,
                            stop=first,
                            skip_group_check=True,
                        )
                        nc.tensor.matmul(
                            ps_n,
                            idw_sb,
                            bhn_sb,
                            start=True,
                            stop=first,
                            skip_group_check=True,
                        )

                        def _mm(slot, wsb, k, mov, stop):
                            out = ps_rz[:, slot] if slot < 4 else ps_n[:, slot - 4]
                            nc.tensor.matmul(
                                out,
                                wsb[:, k, slot],
                                mov[:, k],
                                start=False,
                                stop=stop,
                                skip_group_check=True,
                            )

                        if not first:
                            # h(t-1) = hz_prev - nzm_prev, fed as two moving
                            # operands; nzm pass uses negated weights.  hz
                            # pass executes early (hz ready right after
                            # sigma); nzm pass is the critical tail, rz
                            # slots first so sigma fires before the n slots
                            # finish.
                            for m in range(M3):
                                for k in range(KH):
                                    _mm(m, whh_sb, k, hz_prev, False)
                            for m in (0, 1, 2, 3, 4, 5):
                                for k in range(KH):
                                    _mm(m, whhN_sb, k, nzm_prev, k == KH - 1)
                        # r,z = sigmoid(psum rz tile) in one ACT op
                        rz = spool.tile([128, 4, BL], f32, tag="rz")
                        nc.scalar.activation(rz, ps_rz, AF.Sigmoid)
                        if DEBUG_HS and t == 1:
                            nc.sync.dma_start(
                                out=rz_dbg[:, :],
                                in_=rz.rearrange("p m b -> p (m b)"),
                            )
                        # hz = z * h(t-1) — off critical path, feeds next MMs
                        hz_t = hpool.tile([128, KH, BL], wdt, tag="hz")
                        if not first:
                            nc.gpsimd.tensor_mul(
                                hz_t, rz[:, 2:4], hs_sb[:, :, :, t - 1]
                            )
                        # n = tanh(gx_n + r*(gh_n + b_hn));  b_hn already in psum
                        rn = spool.tile([128, KH, BL], f32, tag="rn")
                        nc.vector.tensor_mul(rn, ps_n, rz[:, 0:2])
                        npre = spool.tile([128, KH, BL], f32, tag="npre")
                        nc.vector.tensor_add(
                            npre,
                            rn,
                            gx[:, 4:6, :, :, tcx].rearrange("p m q b -> p m (q b)"),
                        )
                        n_sb = spool.tile([128, KH, BL], f32, tag="n_sb")
                        nc.scalar.activation(n_sb, npre, AF.Tanh)
                        # nzm = (z-1)*n  (so h = hz - nzm)
                        nzm_t = hpool.tile([128, KH, BL], wdt, tag="nzm")
                        nc.vector.scalar_tensor_tensor(
                            nzm_t,
                            rz[:, 2:4],
                            1.0,
                            n_sb,
                            op0=ALU.subtract,
                            op1=ALU.mult,
                        )
                        # materialize h into history (off critical path)
                        if first:
                            nc.gpsimd.tensor_scalar_mul(
                                hs_sb[:, :, :, t], nzm_t, -1.0
                            )
                        else:
                            nc.gpsimd.tensor_sub(
                                hs_sb[:, :, :, t], hz_t, nzm_t
                            )
                        if first:
                            # t=0: h = -nzm only; next step's hz pass runs
                            # against an explicit zero tile
                            nc.gpsimd.memset(hz_t, 0.0)
                        hz_prev = hz_t
                        nzm_prev = nzm_t

            if DEBUG_HS:
                nc.sync.dma_start(
                    out=hs_dbg[:, :],
                    in_=hs_sb.rearrange("p k b t -> p (k b t)"),
                )
            # ---------------- attention ----------------
            with (
                tc.tile_pool(name="att", bufs=1) as apool,
                tc.tile_pool(name="scr2", bufs=2) as s2pool,
                tc.tile_pool(name="psa", bufs=2, space="PSUM") as psap,
                tc.tile_pool(name="psb", bufs=3, space="PSUM") as psbp,
                tc.tile_pool(name="pss", bufs=1, space="PSUM") as pssp,
            ):
                BT = BL * T_
                nj = BT // 512
                # um = tanh(wv_W @ hs + wv_b): [A, (b,t)] fp16
                um_sb = apool.tile([A, BL, T_], f16)
                umf = um_sb.rearrange("a b t -> a (b t)")
                hsf = hs_sb.rearrange("p k b t -> p k (b t)")
                for j in range(nj):
                    ps_um = psap.tile([A, 512], f32, tag="ps_um")
                    for k in range(KH):
                        nc.tensor.matmul(
                            ps_um,
                            wv_sb[:, k],
                            hsf[:, k, j * 512 : (j + 1) * 512],
                            start=(k == 0),
                            stop=(k == KH - 1),
                        )
                    nc.scalar.activation(
                        umf[:, j * 512 : (j + 1) * 512], ps_um, AF.Tanh, bias=wvb_sb
                    )
                # s[b, t] = sum_a wu[a] um[a, b, t]  -> psum [BL, T]
                ps_s = pssp.tile([BL, T_], f32)
                nth = T_ // 512 if T_ >= 512 else 1
                tsz = min(512, T_)
                for th in range(nth):
                    for b in range(BL):
                        nc.tensor.matmul(
                            ps_s[:, th * tsz : (th + 1) * tsz],
                            wud_sb[:, b],
                            um_sb[:, b, th * tsz : (th + 1) * tsz],
                            start=(b == 0),
                            stop=(b == BL - 1),
                        )
                # softmax over t (free dim)
                nm = s2pool.tile([BL, 1], f32)
                nc.vector.reduce_max(nm, ps_s, axis=AX.X, negate=True)
                expw = s2pool.tile([BL, T_], f32)
                se = s2pool.tile([BL, 1], f32)
                nc.scalar.activation(expw, ps_s, AF.Exp, bias=nm, accum_out=se)
                rse = s2pool.tile([BL, 1], f32)
                nc.vector.reciprocal(rse, se)
                alpha = s2pool.tile([BL, T_], f16)
                nc.vector.tensor_scalar_mul(alpha, expw, rse)
                # context: ctxT[p, k, b] = sum_t hs[p,k,b,t] * alpha[b,t].
                # alpha row b broadcast to 128 partitions via sel matmul; the
                # fused multiply+reduce reads the psum directly.
                ctxT = apool.tile([128, KH, BL], f32)
                for b in range(BL):
                    parts = [[None] * nth for _ in range(KH)]
                    for th in range(nth):
                        ps_ab = psbp.tile([128, tsz], f32, tag="ps_ab")
                        nc.tensor.matmul(
                            ps_ab,
                            sel_sb[:, b],
                            alpha[:, th * tsz : (th + 1) * tsz],
                            start=True,
                            stop=True,
                        )
                        if USE_TTR:
                            for k in range(KH):
                                scr = s2pool.tile([128, tsz], f32, tag="ttr_scr")
                                part = s2pool.tile(
                                    [128, 1], f32, tag="ttr_acc", bufs=8
                                )
                                nc.vector.tensor_tensor_reduce(
                                    out=scr,
                                    in0=hs_sb[:, k, b, th * tsz : (th + 1) * tsz],
                                    in1=ps_ab,
                                    scale=1.0,
                                    scalar=0.0,
                                    op0=ALU.mult,
                                    op1=ALU.add,
                                    accum_out=part,
                                )
                                parts[k][th] = part
                        else:
                            ab = s2pool.tile([128, tsz], f16, tag="ab_sb")
                            ceng = nc.vector if b % 2 == 0 else nc.scalar
                            if ceng is nc.vector:
                                nc.vector.tensor_copy(out=ab, in_=ps_ab)
                            else:
                                nc.scalar.copy(out=ab, in_=ps_ab)
                            for k in range(KH):
                                scr = s2pool.tile([128, tsz], f32, tag="ttr_scr")
                                nc.vector.tensor_mul(
                                    scr,
                                    hs_sb[:, k, b, th * tsz : (th + 1) * tsz],
                                    ab,
                                )
                                part = s2pool.tile(
                                    [128, 1], f32, tag="ttr_acc", bufs=8
                                )
                                nc.vector.reduce_sum(part, scr, axis=AX.X)
                                parts[k][th] = part
                    for k in range(KH):
                        if nth == 1:
                            nc.vector.tensor_copy(
                                out=ctxT[:, k, b : b + 1], in_=parts[k][0]
                            )
                        else:
                            nc.vector.tensor_add(
                                ctxT[:, k, b : b + 1], parts[k][0], parts[k][1]
                            )
                # out = h2o_W . ctx + h2o_b
                ps_o = pssp.tile([1, BL], f32, tag="ps_o")
                for k in range(KH):
                    nc.tensor.matmul(
                        ps_o,
                        h2o_sb[:, k : k + 1],
                        ctxT[:, k],
                        start=(k == 0),
                        stop=(k == KH - 1),
                    )
                o_sb = s2pool.tile([1, BL], f32)
                nc.vector.tensor_scalar_add(o_sb, ps_o, h2ob_sb)
                nc.sync.dma_start(
                    out=out_ext[:, :].rearrange("b one -> one b"), in_=o_sb
                )
    nc.compile()
    return nc


def _prep_maps(inputs, T_):
    x = np.ascontiguousarray(np.asarray(inputs["x"], dtype=np.float32)[:, :T_, :])
    W_ih = np.asarray(inputs["W_ih"], dtype=np.float32)
    W_hh = np.asarray(inputs["W_hh"], dtype=np.float32)
    b_ih = np.asarray(inputs["b_ih"], dtype=np.float32)
    b_hh = np.asarray(inputs["b_hh"], dtype=np.float32)
    wv_W = np.asarray(inputs["wv_W"], dtype=np.float32)
    wv_b = np.asarray(inputs["wv_b"], dtype=np.float32)
    wu = np.asarray(inputs["wu"], dtype=np.float32)
    h2o_W = np.asarray(inputs["h2o_W"], dtype=np.float32)
    h2o_b = np.asarray(inputs["h2o_b"], dtype=np.float32)

    w_np = np.float16 if WDT_F16 else np.float32
    whhT = np.ascontiguousarray(W_hh.T).astype(w_np)  # [H, 3H]
    whhTN = np.ascontiguousarray(-W_hh.T).astype(w_np)  # [H, 3H] negated
    bhn = b_hh[512:768].reshape(2, 128).T  # [128, KH]
    bhn_pack = np.broadcast_to(
        bhn[:, :, None], (128, 2, BL)
    ).reshape(128, 2 * BL).astype(w_np)
    wihT = np.ascontiguousarray(W_ih.T).astype(w_np)  # [I, 3H]
    bsum = b_ih + b_hh
    bias_pack = np.stack(
        [bsum[0:128], bsum[128:256], bsum[256:384], bsum[384:512],
         b_ih[512:640], b_ih[640:768], b_hh[512:640], b_hh[640:768]],
        axis=1,
    ).astype(np.float32)  # [128, 8]
    identity = np.eye(128, dtype=np.float32)
    wvT = np.ascontiguousarray(wv_W.T).astype(np.float16)  # [H, A]
    wvb_ = wv_b.reshape(A, 1).astype(np.float32)
    wud = (wu[:, None, None] * np.eye(BL, dtype=np.float32)[None]).reshape(
        A, BL * BL
    ).astype(np.float16)
    sel = np.repeat(np.eye(BL, dtype=np.float32), 128, axis=1).astype(np.float16)
    h2o_pack = np.ascontiguousarray(
        h2o_W.reshape(KH, 128).T
    ).astype(np.float32)  # [128, KH]
    h2ob_ = h2o_b.reshape(1, 1).astype(np.float32)

    shared = dict(
        W_hhT=whhT, W_hhTN=whhTN, bhn_pack=bhn_pack,
        W_ihT=wihT, bias_pack=bias_pack, identity=identity,
        wv_WT=wvT, wv_b=wvb_, wu_delta=wud, bcast_sel=sel,
        h2o_pack=h2o_pack, h2o_b=h2ob_,
    )
    maps = []
    for c in range(NCORES):
        m = dict(shared)
        m["x"] = np.ascontiguousarray(x[c * BL : (c + 1) * BL])
        maps.append(m)
    return maps


def _execute(inputs, T_=None, trace=False, tmpdir=None, nc=None):
    T_ = T_ or int(os.environ.get("RNN_T", T))
    if nc is None:
        nc = build_program(T_=T_)
    maps = _prep_maps(inputs, T_)
    res = run_bass_kernel_spmd(
        nc, maps, list(range(NCORES)), trace=trace, tmpdir=tmpdir
    )
    out = np.concatenate([res.results[c]["out"] for c in range(NCORES)], axis=0)
    return out.astype(np.float32), res


def kernel(**inputs):
    out, _ = _execute(inputs)
    return out



# revision 3
# speedup vs baseline: 1.0793x; 1.0793x over previous
"""Att_RNN_GRU Trainium2 Bass kernel, latency-optimized recurrence.

Data-parallel over batch: B=128 split across 8 cores (16 each).
Layout strategy: channels-on-partitions, (batch,time)-on-free everywhere.

Chain-minimized GRU step (critical path after the psum matmuls):
  sigma(ps[0:4]) -> rn = ps_n * r -> npre = rn + gx_n -> tanh -> nzm=(z-1)*n
h is kept as the UNMATERIALIZED pair (hz, -nzm): the next step's matmuls
accumulate W*hz (via whh) and W*nzm (via negated weights whhN), so the
h materialization (hs history write, m = hz - nzm) happens off the
critical path on gpsimd.  b_hn enters psum via a constant-tile identity
matmul (inject2); the r,z,n gx enter via inject1 (identity matmul on the
precomputed gx chunk, biases baked).
Attention phase unchanged from baseline (fp16 um/softmax/context).
"""

import os

import numpy as np

import concourse.bass as bass
import concourse.mybir as mybir
from concourse import bacc
from concourse import bass_utils as _bu
from concourse.bass_utils import run_bass_kernel_spmd

_orig_run_command = _bu.run_command


def _run_command_nobs(cmd, **kw):
    cmd = [
        ("--enable-birsim=false" if c == "--enable-birsim=true" else c) for c in cmd
    ]
    return _orig_run_command(cmd, **kw)


_bu.run_command = _run_command_nobs
from concourse.tile import TileContext

B, T, I, H, A = 128, 1024, 128, 256, 40
NCORES = 8
BL = B // NCORES  # 16 batch rows per core
KH = H // 128  # 2 hidden k-chunks
M3 = 3 * H // 128  # 6 gate m-chunks (r: 0,1  z: 2,3  n: 4,5)
CH = 64  # timesteps per gx chunk

f32 = mybir.dt.float32
f16 = mybir.dt.float16
WDT_F16 = os.environ.get("RNN_WDT", "f16") == "f16"
TWOSUM = os.environ.get("RNN_TWOSUM", "1") == "1"
DEBUG_HS = os.environ.get("RNN_DEBUG_HS", "0") == "1"
USE_TTR = os.environ.get("RNN_TTR", "0") == "1"
wdt = f16 if WDT_F16 else f32

AF = mybir.ActivationFunctionType
ALU = mybir.AluOpType
AX = mybir.AxisListType


def build_program(T_=None, CH_=None):
    T_ = T_ or int(os.environ.get("RNN_T", T))
    CH_ = CH_ or int(os.environ.get("RNN_CH", CH))
    nchunk = T_ // CH_
    assert T_ % CH_ == 0

    nc = bacc.Bacc(
        "TRN2", target_bir_lowering=False, debug=False, num_devices=NCORES
    )
    x_ext = nc.declare_dram_parameter("x", [BL, T_, I], f32, isOutput=False)
    whhT = nc.declare_dram_parameter("W_hhT", [H, 3 * H], wdt, isOutput=False)
    whhTN = nc.declare_dram_parameter("W_hhTN", [H, 3 * H], wdt, isOutput=False)
    bhnp = nc.declare_dram_parameter("bhn_pack", [128, 2 * BL], wdt, isOutput=False)
    wihT = nc.declare_dram_parameter("W_ihT", [I, 3 * H], wdt, isOutput=False)
    biasp = nc.declare_dram_parameter("bias_pack", [128, 8], f32, isOutput=False)
    ident = nc.declare_dram_parameter("identity", [128, 128], f32, isOutput=False)
    wvT = nc.declare_dram_parameter("wv_WT", [H, A], f16, isOutput=False)
    wvb = nc.declare_dram_parameter("wv_b", [A, 1], f32, isOutput=False)
    wud = nc.declare_dram_parameter("wu_delta", [A, BL * BL], f16, isOutput=False)
    sel = nc.declare_dram_parameter("bcast_sel", [BL, BL * 128], f16, isOutput=False)
    h2o = nc.declare_dram_parameter("h2o_pack", [128, KH], f32, isOutput=False)
    h2ob = nc.declare_dram_parameter("h2o_b", [1, 1], f32, isOutput=False)
    out_ext = nc.declare_dram_parameter("out", [BL, 1], f32, isOutput=True)
    hs_dbg = (
        nc.declare_dram_parameter("hs_dbg", [128, KH * BL * T_], f16, isOutput=True)
        if DEBUG_HS
        else None
    )
    rz_dbg = (
        nc.declare_dram_parameter("rz_dbg", [128, 4 * BL], f32, isOutput=True)
        if DEBUG_HS
        else None
    )

    with TileContext(nc) as tc:
        with (
            tc.tile_pool(name="consts", bufs=1) as cpool,
            tc.tile_pool(name="hspool", bufs=1) as hspool,
        ):
            # ---------------- constants ----------------
            whh_sb = cpool.tile([128, KH, M3, 128], wdt)
            for k in range(KH):
                nc.sync.dma_start(
                    out=whh_sb[:, k],
                    in_=whhT[k * 128 : (k + 1) * 128, :].rearrange(
                        "p (m c) -> p m c", m=M3
                    ),
                )
            whhN_sb = cpool.tile([128, KH, M3, 128], wdt)
            for k in range(KH):
                nc.sync.dma_start(
                    out=whhN_sb[:, k],
                    in_=whhTN[k * 128 : (k + 1) * 128, :].rearrange(
                        "p (m c) -> p m c", m=M3
                    ),
                )
            bhn_sb = cpool.tile([128, KH, BL], wdt)
            nc.sync.dma_start(
                out=bhn_sb, in_=bhnp[:, :].rearrange("p (k b) -> p k b", k=KH)
            )
            wih_sb = cpool.tile([128, M3, 128], wdt)
            nc.sync.dma_start(
                out=wih_sb, in_=wihT[:, :].rearrange("p (m c) -> p m c", m=M3)
            )
            bias_sb = cpool.tile([128, 8], f32)
            nc.sync.dma_start(out=bias_sb, in_=biasp[:, :])
            id_sb = cpool.tile([128, 128], f32)
            nc.sync.dma_start(out=id_sb, in_=ident[:, :])
            idw_sb = cpool.tile([128, 128], wdt)
            if WDT_F16:
                nc.vector.tensor_copy(out=idw_sb, in_=id_sb)
            else:
                idw_sb = id_sb
            wv_sb = cpool.tile([128, KH, A], f16)
            for k in range(KH):
                nc.sync.dma_start(out=wv_sb[:, k], in_=wvT[k * 128 : (k + 1) * 128, :])
            wvb_sb = cpool.tile([A, 1], f32)
            nc.sync.dma_start(out=wvb_sb, in_=wvb[:, :])
            wud_sb = cpool.tile([A, BL, BL], f16)
            nc.sync.dma_start(out=wud_sb, in_=wud[:, :].rearrange("a (b c) -> a b c", b=BL))
            sel_sb = cpool.tile([BL, BL, 128], f16)
            nc.sync.dma_start(out=sel_sb, in_=sel[:, :].rearrange("a (b c) -> a b c", b=BL))
            h2o_sb = cpool.tile([128, KH], f32)
            nc.sync.dma_start(out=h2o_sb, in_=h2o[:, :])
            h2ob_sb = cpool.tile([1, 1], f32)
            nc.sync.dma_start(out=h2ob_sb, in_=h2ob[:, :])

            # hidden-state history, fp16: [128, k, b, t]
            hs_sb = hspool.tile([128, KH, BL, T_], f16)

            # ---------------- recurrence ----------------
            with (
                tc.tile_pool(name="xio", bufs=2) as xpool,
                tc.tile_pool(name="gxp", bufs=2) as gxpool,
                tc.tile_pool(name="state", bufs=6) as hpool,
                tc.tile_pool(name="scr", bufs=4) as spool,
                tc.tile_pool(name="psgr", bufs=2, space="PSUM") as psgr,
                tc.tile_pool(name="psgn", bufs=2, space="PSUM") as psgn,
                tc.tile_pool(name="pst", bufs=2, space="PSUM") as pstp,
                tc.tile_pool(name="psx", bufs=2, space="PSUM") as psxp,
            ):
                hz_prev = None
                nzm_prev = None

                for c in range(nchunk):
                    t0 = c * CH_
                    npair = BL // 2
                    ncol = BL * CH_  # gx cols this chunk
                    # ---- load x chunk, pair-major rows: [(bo t), i]
                    xin = xpool.tile([128, npair * 128], f32, tag="xin")
                    xiv = xin.rearrange("p (q c) -> p q c", q=npair)
                    for p in range(npair):
                        for bo in range(2):
                            nc.sync.dma_start(
                                out=xiv[bo * CH_ : (bo + 1) * CH_, p],
                                in_=x_ext[2 * p + bo, t0 : t0 + CH_, :],
                            )
                    # ---- transpose -> xT [i, (pair, bo, tc)]
                    xT = xpool.tile([128, ncol], wdt, tag="xT")
                    for p in range(npair):
                        ps_t = pstp.tile([128, 128], f32, tag="ps_t")
                        nc.tensor.transpose(ps_t, xiv[:, p], id_sb)
                        nc.scalar.copy(out=xT[:, p * 128 : (p + 1) * 128], in_=ps_t)
                    # ---- gx matmuls: [128, m, pair, bo, tc] with bias baked
                    # slots 0-3: r,z (bias = b_ih+b_hh); slots 4,5: n (bias = b_ih)
                    gx = gxpool.tile([128, M3, npair, 2, CH_], wdt, tag="gx")
                    gxf = gx.rearrange("p m q b t -> p m (q b t)")
                    nh = ncol // 512 if ncol >= 512 else 1
                    nsz = min(512, ncol)
                    for m in range(M3):
                        for j in range(nh):
                            ps_gx = psxp.tile([128, nsz], f32, tag="ps_gx")
                            nc.tensor.matmul(
                                ps_gx,
                                wih_sb[:, m],
                                xT[:, j * nsz : (j + 1) * nsz],
                                start=True,
                                stop=True,
                            )
                            if (m + j) % 2 == 0:
                                nc.vector.tensor_scalar_add(
                                    gxf[:, m, j * nsz : (j + 1) * nsz],
                                    ps_gx,
                                    bias_sb[:, m : m + 1],
                                )
                            else:
                                nc.scalar.activation(
                                    gxf[:, m, j * nsz : (j + 1) * nsz],
                                    ps_gx,
                                    AF.Identity,
                                    bias=bias_sb[:, m : m + 1],
                                )
                    # ---- CH_ recurrence steps
                    for tcx in range(CH_):
                        t = t0 + tcx
                        first = t == 0
                        # separate psum tiles (= separate banks/semaphores):
                        # sigma waits only on the rz writers, not the n-gate's
                        ps_rz = psgr.tile([128, 4, BL], f32, tag="ps_rz")
                        ps_n = psgn.tile([128, KH, BL], f32, tag="ps_n")
                        nc.tensor.matmul(
                            ps_rz,
                            idw_sb,
                            gx[:, 0:4, :, :, tcx],
                            start=True,
                            stop=first,
                            skip_group_check=True,
                        )
                        nc.tensor.matmul(
                            ps_n,
                            idw_sb,
                            bhn_sb,
                            start=True,
                            stop=first,
                            skip_group_check=True,
                        )

                        def _mm(slot, wsb, k, mov, stop):
                            out = ps_rz[:, slot] if slot < 4 else ps_n[:, slot - 4]
                            nc.tensor.matmul(
                                out,
                                wsb[:, k, slot],
                                mov[:, k],
                                start=False,
                                stop=stop,
                                skip_group_check=True,
                            )

                        if not first:
                            # h(t-1) = hz_prev - nzm_prev, fed as two moving
                            # operands; nzm pass uses negated weights.  hz
                            # pass executes early (hz ready right after
                            # sigma); nzm pass is the critical tail, rz
                            # slots first so sigma fires before the n slots
                            # finish.
                            for m in range(M3):
                                for k in range(KH):
                                    _mm(m, whh_sb, k, hz_prev, False)
                            for m in (0, 1, 2, 3, 4, 5):
                                for k in range(KH):
                                    _mm(m, whhN_sb, k, nzm_prev, k == KH - 1)
                        # r,z = sigmoid(psum rz tile) in one ACT op
                        rz = spool.tile([128, 4, BL], f32, tag="rz")
                        nc.scalar.activation(rz, ps_rz, AF.Sigmoid)
                        if DEBUG_HS and t == 1:
                            nc.sync.dma_start(
                                out=rz_dbg[:, :],
                                in_=rz.rearrange("p m b -> p (m b)"),
                            )
                        # hz = z * h(t-1) — off critical path, feeds next MMs
                        hz_t = hpool.tile([128, KH, BL], wdt, tag="hz")
                        if not first:
                            nc.gpsimd.tensor_mul(
                                hz_t, rz[:, 2:4], hs_sb[:, :, :, t - 1]
                            )
                        # n = tanh(gx_n + r*(gh_n + b_hn));  b_hn already in psum
                        rn = spool.tile([128, KH, BL], f32, tag="rn")
                        nc.vector.tensor_mul(rn, ps_n, rz[:, 0:2])
                        npre = spool.tile([128, KH, BL], f32, tag="npre")
                        nc.vector.tensor_add(
                            npre,
                            rn,
                            gx[:, 4:6, :, :, tcx].rearrange("p m q b -> p m (q b)"),
                        )
                        n_sb = spool.tile([128, KH, BL], f32, tag="n_sb")
                        nc.scalar.activation(n_sb, npre, AF.Tanh)
                        # nzm = (z-1)*n  (so h = hz - nzm)
                        nzm_t = hpool.tile([128, KH, BL], wdt, tag="nzm")
                        nc.vector.scalar_tensor_tensor(
                            nzm_t,
                            rz[:, 2:4],
                            1.0,
                            n_sb,
                            op0=ALU.subtract,
                            op1=ALU.mult,
                        )
                        # materialize h into history (off critical path)
                        if first:
                            nc.gpsimd.tensor_scalar_mul(
                                hs_sb[:, :, :, t], nzm_t, -1.0
                            )
                        else:
                            nc.gpsimd.tensor_sub(
                                hs_sb[:, :, :, t], hz_t, nzm_t
                            )
                        if first:
                            # t=0: h = -nzm only; next step's hz pass runs
                            # against an explicit zero tile
                            nc.gpsimd.memset(hz_t, 0.0)
                        hz_prev = hz_t
                        nzm_prev = nzm_t

            if DEBUG_HS:
                nc.sync.dma_start(
                    out=hs_dbg[:, :],
                    in_=hs_sb.rearrange("p k b t -> p (k b t)"),
                )
            # ---------------- attention ----------------
            with (
                tc.tile_pool(name="att", bufs=1) as apool,
                tc.tile_pool(name="scr2", bufs=2) as s2pool,
                tc.tile_pool(name="psa", bufs=2, space="PSUM") as psap,
                tc.tile_pool(name="psb", bufs=3, space="PSUM") as psbp,
                tc.tile_pool(name="pss", bufs=1, space="PSUM") as pssp,
            ):
                BT = BL * T_
                nj = BT // 512
                # um = tanh(wv_W @ hs + wv_b): [A, (b,t)] fp16
                um_sb = apool.tile([A, BL, T_], f16)
                umf = um_sb.rearrange("a b t -> a (b t)")
                hsf = hs_sb.rearrange("p k b t -> p k (b t)")
                for j in range(nj):
                    ps_um = psap.tile([A, 512], f32, tag="ps_um")
                    for k in range(KH):
                        nc.tensor.matmul(
                            ps_um,
                            wv_sb[:, k],
                            hsf[:, k, j * 512 : (j + 1) * 512],
                            start=(k == 0),
                            stop=(k == KH - 1),
                        )
                    nc.scalar.activation(
                        umf[:, j * 512 : (j + 1) * 512], ps_um, AF.Tanh, bias=wvb_sb
                    )
                # s[b, t] = sum_a wu[a] um[a, b, t]  -> psum [BL, T]
                ps_s = pssp.tile([BL, T_], f32)
                nth = T_ // 512 if T_ >= 512 else 1
                tsz = min(512, T_)
                for th in range(nth):
                    for b in range(BL):
                        nc.tensor.matmul(
                            ps_s[:, th * tsz : (th + 1) * tsz],
                            wud_sb[:, b],
                            um_sb[:, b, th * tsz : (th + 1) * tsz],
                            start=(b == 0),
                            stop=(b == BL - 1),
                        )
                # softmax over t (free dim)
                nm = s2pool.tile([BL, 1], f32)
                nc.vector.reduce_max(nm, ps_s, axis=AX.X, negate=True)
                expw = s2pool.tile([BL, T_], f32)
                se = s2pool.tile([BL, 1], f32)
                nc.scalar.activation(expw, ps_s, AF.Exp, bias=nm, accum_out=se)
                rse = s2pool.tile([BL, 1], f32)
                nc.vector.reciprocal(rse, se)
                alpha = s2pool.tile([BL, T_], f16)
                nc.vector.tensor_scalar_mul(alpha, expw, rse)
                # context: ctxT[p, k, b] = sum_t hs[p,k,b,t] * alpha[b,t].
                # alpha row b broadcast to 128 partitions via sel matmul; the
                # fused multiply+reduce reads the psum directly.
                ctxT = apool.tile([128, KH, BL], f32)
                for b in range(BL):
                    parts = [[None] * nth for _ in range(KH)]
                    for th in range(nth):
                        ps_ab = psbp.tile([128, tsz], f32, tag="ps_ab")
                        nc.tensor.matmul(
                            ps_ab,
                            sel_sb[:, b],
                            alpha[:, th * tsz : (th + 1) * tsz],
                            start=True,
                            stop=True,
                        )
                        if USE_TTR:
                            for k in range(KH):
                                scr = s2pool.tile([128, tsz], f32, tag="ttr_scr")
                                part = s2pool.tile(
                                    [128, 1], f32, tag="ttr_acc", bufs=8
                                )
                                nc.vector.tensor_tensor_reduce(
                                    out=scr,
                                    in0=hs_sb[:, k, b, th * tsz : (th + 1) * tsz],
                                    in1=ps_ab,
                                    scale=1.0,
                                    scalar=0.0,
                                    op0=ALU.mult,
                                    op1=ALU.add,
                                    accum_out=part,
                                )
                                parts[k][th] = part
                        else:
                            ab = s2pool.tile([128, tsz], f16, tag="ab_sb")
                            ceng = nc.vector if b % 2 == 0 else nc.scalar
                            if ceng is nc.vector:
                                nc.vector.tensor_copy(out=ab, in_=ps_ab)
                            else:
                                nc.scalar.copy(out=ab, in_=ps_ab)
                            for k in range(KH):
                                # f16 scratch keeps both DVE ops in 2x mode
                                scr = s2pool.tile([128, tsz], f16, tag="ttr_scr")
                                nc.vector.tensor_mul(
                                    scr,
                                    hs_sb[:, k, b, th * tsz : (th + 1) * tsz],
                                    ab,
                                )
                                part = s2pool.tile(
                                    [128, 1], f32, tag="ttr_acc", bufs=8
                                )
                                nc.vector.reduce_sum(part, scr, axis=AX.X)
                                parts[k][th] = part
                    for k in range(KH):
                        if nth == 1:
                            nc.vector.tensor_copy(
                                out=ctxT[:, k, b : b + 1], in_=parts[k][0]
                            )
                        else:
                            nc.vector.tensor_add(
                                ctxT[:, k, b : b + 1], parts[k][0], parts[k][1]
                            )
                # out = h2o_W . ctx + h2o_b
                ps_o = pssp.tile([1, BL], f32, tag="ps_o")
                for k in range(KH):
                    nc.tensor.matmul(
                        ps_o,
                        h2o_sb[:, k : k + 1],
                        ctxT[:, k],
                        start=(k == 0),
                        stop=(k == KH - 1),
                    )
                o_sb = s2pool.tile([1, BL], f32)
                nc.vector.tensor_scalar_add(o_sb, ps_o, h2ob_sb)
                nc.sync.dma_start(
                    out=out_ext[:, :].rearrange("b one -> one b"), in_=o_sb
                )
    nc.compile()
    return nc


def _prep_maps(inputs, T_):
    x = np.ascontiguousarray(np.asarray(inputs["x"], dtype=np.float32)[:, :T_, :])
    W_ih = np.asarray(inputs["W_ih"], dtype=np.float32)
    W_hh = np.asarray(inputs["W_hh"], dtype=np.float32)
    b_ih = np.asarray(inputs["b_ih"], dtype=np.float32)
    b_hh = np.asarray(inputs["b_hh"], dtype=np.float32)
    wv_W = np.asarray(inputs["wv_W"], dtype=np.float32)
    wv_b = np.asarray(inputs["wv_b"], dtype=np.float32)
    wu = np.asarray(inputs["wu"], dtype=np.float32)
    h2o_W = np.asarray(inputs["h2o_W"], dtype=np.float32)
    h2o_b = np.asarray(inputs["h2o_b"], dtype=np.float32)

    w_np = np.float16 if WDT_F16 else np.float32
    whhT = np.ascontiguousarray(W_hh.T).astype(w_np)  # [H, 3H]
    whhTN = np.ascontiguousarray(-W_hh.T).astype(w_np)  # [H, 3H] negated
    bhn = b_hh[512:768].reshape(2, 128).T  # [128, KH]
    bhn_pack = np.broadcast_to(
        bhn[:, :, None], (128, 2, BL)
    ).reshape(128, 2 * BL).astype(w_np)
    wihT = np.ascontiguousarray(W_ih.T).astype(w_np)  # [I, 3H]
    bsum = b_ih + b_hh
    bias_pack = np.stack(
        [bsum[0:128], bsum[128:256], bsum[256:384], bsum[384:512],
         b_ih[512:640], b_ih[640:768], b_hh[512:640], b_hh[640:768]],
        axis=1,
    ).astype(np.float32)  # [128, 8]
    identity = np.eye(128, dtype=np.float32)
    wvT = np.ascontiguousarray(wv_W.T).astype(np.float16)  # [H, A]
    wvb_ = wv_b.reshape(A, 1).astype(np.float32)
    wud = (wu[:, None, None] * np.eye(BL, dtype=np.float32)[None]).reshape(
        A, BL * BL
    ).astype(np.float16)
    sel = np.repeat(np.eye(BL, dtype=np.float32), 128, axis=1).astype(np.float16)
    h2o_pack = np.ascontiguousarray(
        h2o_W.reshape(KH, 128).T
    ).astype(np.float32)  # [128, KH]
    h2ob_ = h2o_b.reshape(1, 1).astype(np.float32)

    shared = dict(
        W_hhT=whhT, W_hhTN=whhTN, bhn_pack=bhn_pack,
        W_ihT=wihT, bias_pack=bias_pack, identity=identity,
        wv_WT=wvT, wv_b=wvb_, wu_delta=wud, bcast_sel=sel,
        h2o_pack=h2o_pack, h2o_b=h2ob_,
    )
    maps = []
    for c in range(NCORES):
        m = dict(shared)
        m["x"] = np.ascontiguousarray(x[c * BL : (c + 1) * BL])
        maps.append(m)
    return maps


def _execute(inputs, T_=None, trace=False, tmpdir=None, nc=None):
    T_ = T_ or int(os.environ.get("RNN_T", T))
    if nc is None:
        nc = build_program(T_=T_)
    maps = _prep_maps(inputs, T_)
    res = run_bass_kernel_spmd(
        nc, maps, list(range(NCORES)), trace=trace, tmpdir=tmpdir
    )
    out = np.concatenate([res.results[c]["out"] for c in range(NCORES)], axis=0)
    return out.astype(np.float32), res


def kernel(**inputs):
    out, _ = _execute(inputs)
    return out



# revision 4
# speedup vs baseline: 1.0898x; 1.0098x over previous
"""Att_RNN_GRU Trainium2 Bass kernel, latency-optimized recurrence.

Data-parallel over batch: B=128 split across 8 cores (16 each).
Layout strategy: channels-on-partitions, (batch,time)-on-free everywhere.

Chain-minimized GRU step (critical path after the psum matmuls):
  sigma(ps[0:4]) -> rn = ps_n * r -> npre = rn + gx_n -> tanh -> nzm=(z-1)*n
h is kept as the UNMATERIALIZED pair (hz, -nzm): the next step's matmuls
accumulate W*hz (via whh) and W*nzm (via negated weights whhN), so the
h materialization (hs history write, m = hz - nzm) happens off the
critical path on gpsimd.  b_hn enters psum via a constant-tile identity
matmul (inject2); the r,z,n gx enter via inject1 (identity matmul on the
precomputed gx chunk, biases baked).
Attention phase unchanged from baseline (fp16 um/softmax/context).
"""

import os

import numpy as np

import concourse.bass as bass
import concourse.mybir as mybir
from concourse import bacc
from concourse import bass_utils as _bu
from concourse.bass_utils import run_bass_kernel_spmd

_orig_run_command = _bu.run_command


def _run_command_nobs(cmd, **kw):
    cmd = [
        ("--enable-birsim=false" if c == "--enable-birsim=true" else c) for c in cmd
    ]
    return _orig_run_command(cmd, **kw)


_bu.run_command = _run_command_nobs
from concourse.tile import TileContext

B, T, I, H, A = 128, 1024, 128, 256, 40
NCORES = 8
BL = B // NCORES  # 16 batch rows per core
KH = H // 128  # 2 hidden k-chunks
M3 = 3 * H // 128  # 6 gate m-chunks (r: 0,1  z: 2,3  n: 4,5)
CH = 64  # timesteps per gx chunk

f32 = mybir.dt.float32
f16 = mybir.dt.float16
WDT_F16 = os.environ.get("RNN_WDT", "f16") == "f16"
TWOSUM = os.environ.get("RNN_TWOSUM", "1") == "1"
DEBUG_HS = os.environ.get("RNN_DEBUG_HS", "0") == "1"
USE_TTR = os.environ.get("RNN_TTR", "0") == "1"
wdt = f16 if WDT_F16 else f32

AF = mybir.ActivationFunctionType
ALU = mybir.AluOpType
AX = mybir.AxisListType


def build_program(T_=None, CH_=None):
    T_ = T_ or int(os.environ.get("RNN_T", T))
    CH_ = CH_ or int(os.environ.get("RNN_CH", CH))
    nchunk = T_ // CH_
    assert T_ % CH_ == 0

    nc = bacc.Bacc(
        "TRN2", target_bir_lowering=False, debug=False, num_devices=NCORES
    )
    x_ext = nc.declare_dram_parameter("x", [BL, T_, I], f32, isOutput=False)
    whhT = nc.declare_dram_parameter("W_hhT", [H, 3 * H], wdt, isOutput=False)
    whhTN = nc.declare_dram_parameter("W_hhTN", [H, 3 * H], wdt, isOutput=False)
    bhnp = nc.declare_dram_parameter("bhn_pack", [128, 2 * BL], wdt, isOutput=False)
    wihT = nc.declare_dram_parameter("W_ihT", [I, 3 * H], wdt, isOutput=False)
    biasp = nc.declare_dram_parameter("bias_pack", [128, 8], f32, isOutput=False)
    ident = nc.declare_dram_parameter("identity", [128, 128], f32, isOutput=False)
    wvT = nc.declare_dram_parameter("wv_WT", [H, A], f16, isOutput=False)
    wvb = nc.declare_dram_parameter("wv_b", [A, 1], f32, isOutput=False)
    wud = nc.declare_dram_parameter("wu_delta", [A, BL * BL], f16, isOutput=False)
    sel = nc.declare_dram_parameter("bcast_sel", [BL, BL * 128], f16, isOutput=False)
    h2o = nc.declare_dram_parameter("h2o_pack", [128, KH], f32, isOutput=False)
    h2ob = nc.declare_dram_parameter("h2o_b", [1, 1], f32, isOutput=False)
    out_ext = nc.declare_dram_parameter("out", [BL, 1], f32, isOutput=True)
    hs_dbg = (
        nc.declare_dram_parameter("hs_dbg", [128, KH * BL * T_], f16, isOutput=True)
        if DEBUG_HS
        else None
    )
    rz_dbg = (
        nc.declare_dram_parameter("rz_dbg", [128, 4 * BL], f32, isOutput=True)
        if DEBUG_HS
        else None
    )

    with TileContext(nc) as tc:
        with (
            tc.tile_pool(name="consts", bufs=1) as cpool,
            tc.tile_pool(name="hspool", bufs=1) as hspool,
        ):
            # ---------------- constants ----------------
            whh_sb = cpool.tile([128, KH, M3, 128], wdt)
            for k in range(KH):
                nc.sync.dma_start(
                    out=whh_sb[:, k],
                    in_=whhT[k * 128 : (k + 1) * 128, :].rearrange(
                        "p (m c) -> p m c", m=M3
                    ),
                )
            whhN_sb = cpool.tile([128, KH, M3, 128], wdt)
            for k in range(KH):
                nc.sync.dma_start(
                    out=whhN_sb[:, k],
                    in_=whhTN[k * 128 : (k + 1) * 128, :].rearrange(
                        "p (m c) -> p m c", m=M3
                    ),
                )
            bhn_sb = cpool.tile([128, KH, BL], wdt)
            nc.sync.dma_start(
                out=bhn_sb, in_=bhnp[:, :].rearrange("p (k b) -> p k b", k=KH)
            )
            wih_sb = cpool.tile([128, M3, 128], wdt)
            nc.sync.dma_start(
                out=wih_sb, in_=wihT[:, :].rearrange("p (m c) -> p m c", m=M3)
            )
            bias_sb = cpool.tile([128, 8], f32)
            nc.sync.dma_start(out=bias_sb, in_=biasp[:, :])
            id_sb = cpool.tile([128, 128], f32)
            nc.sync.dma_start(out=id_sb, in_=ident[:, :])
            idw_sb = cpool.tile([128, 128], wdt)
            if WDT_F16:
                nc.vector.tensor_copy(out=idw_sb, in_=id_sb)
            else:
                idw_sb = id_sb
            wv_sb = cpool.tile([128, KH, A], f16)
            for k in range(KH):
                nc.sync.dma_start(out=wv_sb[:, k], in_=wvT[k * 128 : (k + 1) * 128, :])
            wvb_sb = cpool.tile([A, 1], f32)
            nc.sync.dma_start(out=wvb_sb, in_=wvb[:, :])
            wud_sb = cpool.tile([A, BL, BL], f16)
            nc.sync.dma_start(out=wud_sb, in_=wud[:, :].rearrange("a (b c) -> a b c", b=BL))
            sel_sb = cpool.tile([BL, BL, 128], f16)
            nc.sync.dma_start(out=sel_sb, in_=sel[:, :].rearrange("a (b c) -> a b c", b=BL))
            h2o_sb = cpool.tile([128, KH], f32)
            nc.sync.dma_start(out=h2o_sb, in_=h2o[:, :])
            h2ob_sb = cpool.tile([1, 1], f32)
            nc.sync.dma_start(out=h2ob_sb, in_=h2ob[:, :])

            # hidden-state history, fp16: [128, k, b, t]
            hs_sb = hspool.tile([128, KH, BL, T_], f16)

            # ---------------- recurrence ----------------
            with (
                tc.tile_pool(name="xio", bufs=2) as xpool,
                tc.tile_pool(name="gxp", bufs=2) as gxpool,
                tc.tile_pool(name="state", bufs=6) as hpool,
                tc.tile_pool(name="scr", bufs=4) as spool,
                tc.tile_pool(name="psr", bufs=1, space="PSUM") as psrp,
                tc.tile_pool(name="psz", bufs=1, space="PSUM") as pszp,
                tc.tile_pool(name="psgn", bufs=2, space="PSUM") as psgn,
                tc.tile_pool(name="pst", bufs=2, space="PSUM") as pstp,
                tc.tile_pool(name="psx", bufs=2, space="PSUM") as psxp,
            ):
                hz_prev = None
                nzm_prev = None

                for c in range(nchunk):
                    t0 = c * CH_
                    npair = BL // 2
                    ncol = BL * CH_  # gx cols this chunk
                    # ---- load x chunk, pair-major rows: [(bo t), i]
                    xin = xpool.tile([128, npair * 128], f32, tag="xin")
                    xiv = xin.rearrange("p (q c) -> p q c", q=npair)
                    for p in range(npair):
                        for bo in range(2):
                            nc.sync.dma_start(
                                out=xiv[bo * CH_ : (bo + 1) * CH_, p],
                                in_=x_ext[2 * p + bo, t0 : t0 + CH_, :],
                            )
                    # ---- transpose -> xT [i, (pair, bo, tc)]
                    xT = xpool.tile([128, ncol], wdt, tag="xT")
                    for p in range(npair):
                        ps_t = pstp.tile([128, 128], f32, tag="ps_t")
                        nc.tensor.transpose(ps_t, xiv[:, p], id_sb)
                        nc.scalar.copy(out=xT[:, p * 128 : (p + 1) * 128], in_=ps_t)
                    # ---- gx matmuls: [128, m, pair, bo, tc] with bias baked
                    # slots 0-3: r,z (bias = b_ih+b_hh); slots 4,5: n (bias = b_ih)
                    gx = gxpool.tile([128, M3, npair, 2, CH_], wdt, tag="gx")
                    gxf = gx.rearrange("p m q b t -> p m (q b t)")
                    nh = ncol // 512 if ncol >= 512 else 1
                    nsz = min(512, ncol)
                    for m in range(M3):
                        for j in range(nh):
                            ps_gx = psxp.tile([128, nsz], f32, tag="ps_gx")
                            nc.tensor.matmul(
                                ps_gx,
                                wih_sb[:, m],
                                xT[:, j * nsz : (j + 1) * nsz],
                                start=True,
                                stop=True,
                            )
                            if (m + j) % 2 == 0:
                                nc.vector.tensor_scalar_add(
                                    gxf[:, m, j * nsz : (j + 1) * nsz],
                                    ps_gx,
                                    bias_sb[:, m : m + 1],
                                )
                            else:
                                nc.scalar.activation(
                                    gxf[:, m, j * nsz : (j + 1) * nsz],
                                    ps_gx,
                                    AF.Identity,
                                    bias=bias_sb[:, m : m + 1],
                                )
                    # ---- CH_ recurrence steps
                    for tcx in range(CH_):
                        t = t0 + tcx
                        first = t == 0
                        # three psum tiles (= separate banks/semaphores):
                        # sigma_r fires after only the r-slot writers; z and
                        # n slots finish later, off the critical head.
                        ps_r = psrp.tile([128, 2, BL], f32, tag="ps_r")
                        ps_z = pszp.tile([128, 2, BL], f32, tag="ps_z")
                        ps_n = psgn.tile([128, KH, BL], f32, tag="ps_n")
                        nc.tensor.matmul(
                            ps_r,
                            idw_sb,
                            gx[:, 0:2, :, :, tcx],
                            start=True,
                            stop=first,
                            skip_group_check=True,
                        )
                        nc.tensor.matmul(
                            ps_z,
                            idw_sb,
                            gx[:, 2:4, :, :, tcx],
                            start=True,
                            stop=first,
                            skip_group_check=True,
                        )
                        nc.tensor.matmul(
                            ps_n,
                            idw_sb,
                            bhn_sb,
                            start=True,
                            stop=first,
                            skip_group_check=True,
                        )
                        ps_by_slot = (ps_r, ps_r, ps_z, ps_z, ps_n, ps_n)

                        def _mm(slot, wsb, k, mov, stop):
                            out = ps_by_slot[slot][:, slot % 2]
                            nc.tensor.matmul(
                                out,
                                wsb[:, k, slot],
                                mov[:, k],
                                start=False,
                                stop=stop,
                                skip_group_check=True,
                            )

                        if not first:
                            # h(t-1) = hz_prev - nzm_prev, fed as two moving
                            # operands; nzm pass uses negated weights.  hz
                            # pass executes early (hz ready right after
                            # sigma_z); nzm pass is the critical tail, r
                            # slots first so sigma_r fires after 4 MMs.
                            for m in range(M3):
                                for k in range(KH):
                                    _mm(m, whh_sb, k, hz_prev, False)
                            for m in (0, 1, 2, 3, 4, 5):
                                for k in range(KH):
                                    _mm(m, whhN_sb, k, nzm_prev, k == KH - 1)
                        rz = spool.tile([128, 4, BL], f32, tag="rz")
                        nc.scalar.activation(rz[:, 0:2], ps_r, AF.Sigmoid)
                        nc.scalar.activation(rz[:, 2:4], ps_z, AF.Sigmoid)
                        if DEBUG_HS and t == 1:
                            nc.sync.dma_start(
                                out=rz_dbg[:, :],
                                in_=rz.rearrange("p m b -> p (m b)"),
                            )
                        # hz = z * h(t-1) — off critical path, feeds next MMs
                        hz_t = hpool.tile([128, KH, BL], wdt, tag="hz")
                        if not first:
                            nc.gpsimd.tensor_mul(
                                hz_t, rz[:, 2:4], hs_sb[:, :, :, t - 1]
                            )
                        # n = tanh(gx_n + r*(gh_n + b_hn));  b_hn already in psum
                        rn = spool.tile([128, KH, BL], f32, tag="rn")
                        nc.vector.tensor_mul(rn, ps_n, rz[:, 0:2])
                        npre = spool.tile([128, KH, BL], f32, tag="npre")
                        nc.vector.tensor_add(
                            npre,
                            rn,
                            gx[:, 4:6, :, :, tcx].rearrange("p m q b -> p m (q b)"),
                        )
                        n_sb = spool.tile([128, KH, BL], f32, tag="n_sb")
                        nc.scalar.activation(n_sb, npre, AF.Tanh)
                        # nzm = (z-1)*n  (so h = hz - nzm)
                        nzm_t = hpool.tile([128, KH, BL], wdt, tag="nzm")
                        nc.vector.scalar_tensor_tensor(
                            nzm_t,
                            rz[:, 2:4],
                            1.0,
                            n_sb,
                            op0=ALU.subtract,
                            op1=ALU.mult,
                        )
                        # materialize h into history (off critical path)
                        if first:
                            nc.gpsimd.tensor_scalar_mul(
                                hs_sb[:, :, :, t], nzm_t, -1.0
                            )
                        else:
                            nc.gpsimd.tensor_sub(
                                hs_sb[:, :, :, t], hz_t, nzm_t
                            )
                        if first:
                            # t=0: h = -nzm only; next step's hz pass runs
                            # against an explicit zero tile
                            nc.gpsimd.memset(hz_t, 0.0)
                        hz_prev = hz_t
                        nzm_prev = nzm_t

            if DEBUG_HS:
                nc.sync.dma_start(
                    out=hs_dbg[:, :],
                    in_=hs_sb.rearrange("p k b t -> p (k b t)"),
                )
            # ---------------- attention ----------------
            with (
                tc.tile_pool(name="att", bufs=1) as apool,
                tc.tile_pool(name="scr2", bufs=2) as s2pool,
                tc.tile_pool(name="psa", bufs=2, space="PSUM") as psap,
                tc.tile_pool(name="psb", bufs=3, space="PSUM") as psbp,
                tc.tile_pool(name="pss", bufs=1, space="PSUM") as pssp,
            ):
                BT = BL * T_
                nj = BT // 512
                # um = tanh(wv_W @ hs + wv_b): [A, (b,t)] fp16
                um_sb = apool.tile([A, BL, T_], f16)
                umf = um_sb.rearrange("a b t -> a (b t)")
                hsf = hs_sb.rearrange("p k b t -> p k (b t)")
                for j in range(nj):
                    ps_um = psap.tile([A, 512], f32, tag="ps_um")
                    for k in range(KH):
                        nc.tensor.matmul(
                            ps_um,
                            wv_sb[:, k],
                            hsf[:, k, j * 512 : (j + 1) * 512],
                            start=(k == 0),
                            stop=(k == KH - 1),
                        )
                    nc.scalar.activation(
                        umf[:, j * 512 : (j + 1) * 512], ps_um, AF.Tanh, bias=wvb_sb
                    )
                # s[b, t] = sum_a wu[a] um[a, b, t]  -> psum [BL, T]
                ps_s = pssp.tile([BL, T_], f32)
                nth = T_ // 512 if T_ >= 512 else 1
                tsz = min(512, T_)
                for th in range(nth):
                    for b in range(BL):
                        nc.tensor.matmul(
                            ps_s[:, th * tsz : (th + 1) * tsz],
                            wud_sb[:, b],
                            um_sb[:, b, th * tsz : (th + 1) * tsz],
                            start=(b == 0),
                            stop=(b == BL - 1),
                        )
                # softmax over t (free dim)
                nm = s2pool.tile([BL, 1], f32)
                nc.vector.reduce_max(nm, ps_s, axis=AX.X, negate=True)
                expw = s2pool.tile([BL, T_], f32)
                se = s2pool.tile([BL, 1], f32)
                nc.scalar.activation(expw, ps_s, AF.Exp, bias=nm, accum_out=se)
                rse = s2pool.tile([BL, 1], f32)
                nc.vector.reciprocal(rse, se)
                alpha = s2pool.tile([BL, T_], f16)
                nc.vector.tensor_scalar_mul(alpha, expw, rse)
                # context: ctxT[p, k, b] = sum_t hs[p,k,b,t] * alpha[b,t].
                # alpha row b broadcast to 128 partitions via sel matmul; the
                # fused multiply+reduce reads the psum directly.
                ctxT = apool.tile([128, KH, BL], f32)
                for b in range(BL):
                    parts = [[None] * nth for _ in range(KH)]
                    for th in range(nth):
                        ps_ab = psbp.tile([128, tsz], f32, tag="ps_ab")
                        nc.tensor.matmul(
                            ps_ab,
                            sel_sb[:, b],
                            alpha[:, th * tsz : (th + 1) * tsz],
                            start=True,
                            stop=True,
                        )
                        if USE_TTR:
                            for k in range(KH):
                                scr = s2pool.tile([128, tsz], f32, tag="ttr_scr")
                                part = s2pool.tile(
                                    [128, 1], f32, tag="ttr_acc", bufs=8
                                )
                                nc.vector.tensor_tensor_reduce(
                                    out=scr,
                                    in0=hs_sb[:, k, b, th * tsz : (th + 1) * tsz],
                                    in1=ps_ab,
                                    scale=1.0,
                                    scalar=0.0,
                                    op0=ALU.mult,
                                    op1=ALU.add,
                                    accum_out=part,
                                )
                                parts[k][th] = part
                        else:
                            ab = s2pool.tile([128, tsz], f16, tag="ab_sb")
                            ceng = nc.vector if b % 2 == 0 else nc.scalar
                            if ceng is nc.vector:
                                nc.vector.tensor_copy(out=ab, in_=ps_ab)
                            else:
                                nc.scalar.copy(out=ab, in_=ps_ab)
                            for k in range(KH):
                                # f16 scratch keeps both DVE ops in 2x mode
                                scr = s2pool.tile([128, tsz], f16, tag="ttr_scr")
                                nc.vector.tensor_mul(
                                    scr,
                                    hs_sb[:, k, b, th * tsz : (th + 1) * tsz],
                                    ab,
                                )
                                part = s2pool.tile(
                                    [128, 1], f32, tag="ttr_acc", bufs=8
                                )
                                nc.vector.reduce_sum(part, scr, axis=AX.X)
                                parts[k][th] = part
                    for k in range(KH):
                        if nth == 1:
                            nc.vector.tensor_copy(
                                out=ctxT[:, k, b : b + 1], in_=parts[k][0]
                            )
                        else:
                            nc.vector.tensor_add(
                                ctxT[:, k, b : b + 1], parts[k][0], parts[k][1]
                            )
                # out = h2o_W . ctx + h2o_b
                ps_o = pssp.tile([1, BL], f32, tag="ps_o")
                for k in range(KH):
                    nc.tensor.matmul(
                        ps_o,
                        h2o_sb[:, k : k + 1],
                        ctxT[:, k],
                        start=(k == 0),
                        stop=(k == KH - 1),
                    )
                o_sb = s2pool.tile([1, BL], f32)
                nc.vector.tensor_scalar_add(o_sb, ps_o, h2ob_sb)
                nc.sync.dma_start(
                    out=out_ext[:, :].rearrange("b one -> one b"), in_=o_sb
                )
    nc.compile()
    return nc


def _prep_maps(inputs, T_):
    x = np.ascontiguousarray(np.asarray(inputs["x"], dtype=np.float32)[:, :T_, :])
    W_ih = np.asarray(inputs["W_ih"], dtype=np.float32)
    W_hh = np.asarray(inputs["W_hh"], dtype=np.float32)
    b_ih = np.asarray(inputs["b_ih"], dtype=np.float32)
    b_hh = np.asarray(inputs["b_hh"], dtype=np.float32)
    wv_W = np.asarray(inputs["wv_W"], dtype=np.float32)
    wv_b = np.asarray(inputs["wv_b"], dtype=np.float32)
    wu = np.asarray(inputs["wu"], dtype=np.float32)
    h2o_W = np.asarray(inputs["h2o_W"], dtype=np.float32)
    h2o_b = np.asarray(inputs["h2o_b"], dtype=np.float32)

    w_np = np.float16 if WDT_F16 else np.float32
    whhT = np.ascontiguousarray(W_hh.T).astype(w_np)  # [H, 3H]
    whhTN = np.ascontiguousarray(-W_hh.T).astype(w_np)  # [H, 3H] negated
    bhn = b_hh[512:768].reshape(2, 128).T  # [128, KH]
    bhn_pack = np.broadcast_to(
        bhn[:, :, None], (128, 2, BL)
    ).reshape(128, 2 * BL).astype(w_np)
    wihT = np.ascontiguousarray(W_ih.T).astype(w_np)  # [I, 3H]
    bsum = b_ih + b_hh
    bias_pack = np.stack(
        [bsum[0:128], bsum[128:256], bsum[256:384], bsum[384:512],
         b_ih[512:640], b_ih[640:768], b_hh[512:640], b_hh[640:768]],
        axis=1,
    ).astype(np.float32)  # [128, 8]
    identity = np.eye(128, dtype=np.float32)
    wvT = np.ascontiguousarray(wv_W.T).astype(np.float16)  # [H, A]
    wvb_ = wv_b.reshape(A, 1).astype(np.float32)
    wud = (wu[:, None, None] * np.eye(BL, dtype=np.float32)[None]).reshape(
        A, BL * BL
    ).astype(np.float16)
    sel = np.repeat(np.eye(BL, dtype=np.float32), 128, axis=1).astype(np.float16)
    h2o_pack = np.ascontiguousarray(
        h2o_W.reshape(KH, 128).T
    ).astype(np.float32)  # [128, KH]
    h2ob_ = h2o_b.reshape(1, 1).astype(np.float32)

    shared = dict(
        W_hhT=whhT, W_hhTN=whhTN, bhn_pack=bhn_pack,
        W_ihT=wihT, bias_pack=bias_pack, identity=identity,
        wv_WT=wvT, wv_b=wvb_, wu_delta=wud, bcast_sel=sel,
        h2o_pack=h2o_pack, h2o_b=h2ob_,
    )
    maps = []
    for c in range(NCORES):
        m = dict(shared)
        m["x"] = np.ascontiguousarray(x[c * BL : (c + 1) * BL])
        maps.append(m)
    return maps


def _execute(inputs, T_=None, trace=False, tmpdir=None, nc=None):
    T_ = T_ or int(os.environ.get("RNN_T", T))
    if nc is None:
        nc = build_program(T_=T_)
    maps = _prep_maps(inputs, T_)
    res = run_bass_kernel_spmd(
        nc, maps, list(range(NCORES)), trace=trace, tmpdir=tmpdir
    )
    out = np.concatenate([res.results[c]["out"] for c in range(NCORES)], axis=0)
    return out.astype(np.float32), res


def kernel(**inputs):
    out, _ = _execute(inputs)
    return out



# revision 5
# speedup vs baseline: 1.0920x; 1.0020x over previous
"""Att_RNN_GRU Trainium2 Bass kernel, latency-optimized recurrence.

Data-parallel over batch: B=128 split across 8 cores (16 each).
Layout strategy: channels-on-partitions, (batch,time)-on-free everywhere.

Chain-minimized GRU step (critical path after the psum matmuls):
  sigma(ps[0:4]) -> rn = ps_n * r -> npre = rn + gx_n -> tanh -> nzm=(z-1)*n
h is kept as the UNMATERIALIZED pair (hz, -nzm): the next step's matmuls
accumulate W*hz (via whh) and W*nzm (via negated weights whhN), so the
h materialization (hs history write, m = hz - nzm) happens off the
critical path on gpsimd.  b_hn enters psum via a constant-tile identity
matmul (inject2); the r,z,n gx enter via inject1 (identity matmul on the
precomputed gx chunk, biases baked).
Attention phase unchanged from baseline (fp16 um/softmax/context).
"""

import os

import numpy as np

import concourse.bass as bass
import concourse.mybir as mybir
from concourse import bacc
from concourse import bass_utils as _bu
from concourse.bass_utils import run_bass_kernel_spmd

_orig_run_command = _bu.run_command


def _run_command_nobs(cmd, **kw):
    cmd = [
        ("--enable-birsim=false" if c == "--enable-birsim=true" else c) for c in cmd
    ]
    return _orig_run_command(cmd, **kw)


_bu.run_command = _run_command_nobs
from concourse.tile import TileContext

B, T, I, H, A = 128, 1024, 128, 256, 40
NCORES = 8
BL = B // NCORES  # 16 batch rows per core
KH = H // 128  # 2 hidden k-chunks
M3 = 3 * H // 128  # 6 gate m-chunks (r: 0,1  z: 2,3  n: 4,5)
CH = 64  # timesteps per gx chunk

f32 = mybir.dt.float32
f16 = mybir.dt.float16
WDT_F16 = os.environ.get("RNN_WDT", "f16") == "f16"
TWOSUM = os.environ.get("RNN_TWOSUM", "1") == "1"
DEBUG_HS = os.environ.get("RNN_DEBUG_HS", "0") == "1"
USE_TTR = os.environ.get("RNN_TTR", "0") == "1"
WARM = os.environ.get("RNN_WARM", "0") == "1"
wdt = f16 if WDT_F16 else f32

AF = mybir.ActivationFunctionType
ALU = mybir.AluOpType
AX = mybir.AxisListType


def build_program(T_=None, CH_=None):
    T_ = T_ or int(os.environ.get("RNN_T", T))
    CH_ = CH_ or int(os.environ.get("RNN_CH", CH))
    nchunk = T_ // CH_
    assert T_ % CH_ == 0

    nc = bacc.Bacc(
        "TRN2", target_bir_lowering=False, debug=False, num_devices=NCORES
    )
    x_ext = nc.declare_dram_parameter("x", [BL, T_, I], f32, isOutput=False)
    whhT = nc.declare_dram_parameter("W_hhT", [H, 3 * H], wdt, isOutput=False)
    whhTN = nc.declare_dram_parameter("W_hhTN", [H, 3 * H], wdt, isOutput=False)
    bhnp = nc.declare_dram_parameter("bhn_pack", [128, 2 * BL], wdt, isOutput=False)
    wihT = nc.declare_dram_parameter("W_ihT", [I, 3 * H], wdt, isOutput=False)
    biasp = nc.declare_dram_parameter("bias_pack", [128, 8], f32, isOutput=False)
    ident = nc.declare_dram_parameter("identity", [128, 128], f32, isOutput=False)
    wvT = nc.declare_dram_parameter("wv_WT", [H, A], f16, isOutput=False)
    wvb = nc.declare_dram_parameter("wv_b", [A, 1], f32, isOutput=False)
    wud = nc.declare_dram_parameter("wu_delta", [A, BL * BL], f16, isOutput=False)
    sel = nc.declare_dram_parameter("bcast_sel", [BL, BL * 128], f16, isOutput=False)
    h2o = nc.declare_dram_parameter("h2o_pack", [128, KH], f32, isOutput=False)
    h2ob = nc.declare_dram_parameter("h2o_b", [1, 1], f32, isOutput=False)
    out_ext = nc.declare_dram_parameter("out", [BL, 1], f32, isOutput=True)
    hs_dbg = (
        nc.declare_dram_parameter("hs_dbg", [128, KH * BL * T_], f16, isOutput=True)
        if DEBUG_HS
        else None
    )
    rz_dbg = (
        nc.declare_dram_parameter("rz_dbg", [128, 4 * BL], f32, isOutput=True)
        if DEBUG_HS
        else None
    )

    with TileContext(nc) as tc:
        with (
            tc.tile_pool(name="consts", bufs=1) as cpool,
            tc.tile_pool(name="hspool", bufs=1) as hspool,
        ):
            # ---------------- constants ----------------
            whh_sb = cpool.tile([128, KH, M3, 128], wdt)
            for k in range(KH):
                nc.sync.dma_start(
                    out=whh_sb[:, k],
                    in_=whhT[k * 128 : (k + 1) * 128, :].rearrange(
                        "p (m c) -> p m c", m=M3
                    ),
                )
            whhN_sb = cpool.tile([128, KH, M3, 128], wdt)
            for k in range(KH):
                nc.sync.dma_start(
                    out=whhN_sb[:, k],
                    in_=whhTN[k * 128 : (k + 1) * 128, :].rearrange(
                        "p (m c) -> p m c", m=M3
                    ),
                )
            bhn_sb = cpool.tile([128, KH, BL], wdt)
            nc.sync.dma_start(
                out=bhn_sb, in_=bhnp[:, :].rearrange("p (k b) -> p k b", k=KH)
            )
            wih_sb = cpool.tile([128, M3, 128], wdt)
            nc.sync.dma_start(
                out=wih_sb, in_=wihT[:, :].rearrange("p (m c) -> p m c", m=M3)
            )
            bias_sb = cpool.tile([128, 8], f32)
            nc.sync.dma_start(out=bias_sb, in_=biasp[:, :])
            id_sb = cpool.tile([128, 128], f32)
            nc.sync.dma_start(out=id_sb, in_=ident[:, :])
            idw_sb = cpool.tile([128, 128], wdt)
            if WDT_F16:
                nc.vector.tensor_copy(out=idw_sb, in_=id_sb)
            else:
                idw_sb = id_sb
            wv_sb = cpool.tile([128, KH, A], f16)
            for k in range(KH):
                nc.sync.dma_start(out=wv_sb[:, k], in_=wvT[k * 128 : (k + 1) * 128, :])
            wvb_sb = cpool.tile([A, 1], f32)
            nc.sync.dma_start(out=wvb_sb, in_=wvb[:, :])
            wud_sb = cpool.tile([A, BL, BL], f16)
            nc.sync.dma_start(out=wud_sb, in_=wud[:, :].rearrange("a (b c) -> a b c", b=BL))
            sel_sb = cpool.tile([BL, BL, 128], f16)
            nc.sync.dma_start(out=sel_sb, in_=sel[:, :].rearrange("a (b c) -> a b c", b=BL))
            h2o_sb = cpool.tile([128, KH], f32)
            nc.sync.dma_start(out=h2o_sb, in_=h2o[:, :])
            h2ob_sb = cpool.tile([1, 1], f32)
            nc.sync.dma_start(out=h2ob_sb, in_=h2ob[:, :])

            # hidden-state history, fp16: [128, k, b, t]
            hs_sb = hspool.tile([128, KH, BL, T_], f16)

            # ---------------- recurrence ----------------
            with (
                tc.tile_pool(name="xio", bufs=2) as xpool,
                tc.tile_pool(name="gxp", bufs=2) as gxpool,
                tc.tile_pool(name="state", bufs=6) as hpool,
                tc.tile_pool(name="scr", bufs=4) as spool,
                tc.tile_pool(name="psr", bufs=1, space="PSUM") as psrp,
                tc.tile_pool(name="psz", bufs=1, space="PSUM") as pszp,
                tc.tile_pool(name="psgn", bufs=2, space="PSUM") as psgn,
                tc.tile_pool(name="pst", bufs=2, space="PSUM") as pstp,
                tc.tile_pool(name="psx", bufs=2, space="PSUM") as psxp,
                tc.tile_pool(name="psw", bufs=1, space="PSUM") as pswp,
            ):
                hz_prev = None
                nzm_prev = None

                for c in range(nchunk):
                    t0 = c * CH_
                    npair = BL // 2
                    ncol = BL * CH_  # gx cols this chunk
                    # ---- load x chunk, pair-major rows: [(bo t), i]
                    xin = xpool.tile([128, npair * 128], f32, tag="xin")
                    xiv = xin.rearrange("p (q c) -> p q c", q=npair)
                    for p in range(npair):
                        for bo in range(2):
                            nc.sync.dma_start(
                                out=xiv[bo * CH_ : (bo + 1) * CH_, p],
                                in_=x_ext[2 * p + bo, t0 : t0 + CH_, :],
                            )
                    # ---- transpose -> xT [i, (pair, bo, tc)]
                    xT = xpool.tile([128, ncol], wdt, tag="xT")
                    for p in range(npair):
                        ps_t = pstp.tile([128, 128], f32, tag="ps_t")
                        nc.tensor.transpose(ps_t, xiv[:, p], id_sb)
                        nc.scalar.copy(out=xT[:, p * 128 : (p + 1) * 128], in_=ps_t)
                    # ---- gx matmuls: [128, m, pair, bo, tc] with bias baked
                    # slots 0-3: r,z (bias = b_ih+b_hh); slots 4,5: n (bias = b_ih)
                    gx = gxpool.tile([128, M3, npair, 2, CH_], wdt, tag="gx")
                    gxf = gx.rearrange("p m q b t -> p m (q b t)")
                    nh = ncol // 512 if ncol >= 512 else 1
                    nsz = min(512, ncol)
                    for m in range(M3):
                        for j in range(nh):
                            ps_gx = psxp.tile([128, nsz], f32, tag="ps_gx")
                            nc.tensor.matmul(
                                ps_gx,
                                wih_sb[:, m],
                                xT[:, j * nsz : (j + 1) * nsz],
                                start=True,
                                stop=True,
                            )
                            if (m + j) % 2 == 0:
                                nc.vector.tensor_scalar_add(
                                    gxf[:, m, j * nsz : (j + 1) * nsz],
                                    ps_gx,
                                    bias_sb[:, m : m + 1],
                                )
                            else:
                                nc.scalar.activation(
                                    gxf[:, m, j * nsz : (j + 1) * nsz],
                                    ps_gx,
                                    AF.Identity,
                                    bias=bias_sb[:, m : m + 1],
                                )
                    # ---- CH_ recurrence steps
                    for tcx in range(CH_):
                        t = t0 + tcx
                        first = t == 0
                        # three psum tiles (= separate banks/semaphores):
                        # sigma_r fires after only the r-slot writers; z and
                        # n slots finish later, off the critical head.
                        ps_r = psrp.tile([128, 2, BL], f32, tag="ps_r")
                        ps_z = pszp.tile([128, 2, BL], f32, tag="ps_z")
                        ps_n = psgn.tile([128, KH, BL], f32, tag="ps_n")
                        nc.tensor.matmul(
                            ps_r,
                            idw_sb,
                            gx[:, 0:2, :, :, tcx],
                            start=True,
                            stop=first,
                            skip_group_check=True,
                        )
                        nc.tensor.matmul(
                            ps_z,
                            idw_sb,
                            gx[:, 2:4, :, :, tcx],
                            start=True,
                            stop=first,
                            skip_group_check=True,
                        )
                        nc.tensor.matmul(
                            ps_n,
                            idw_sb,
                            bhn_sb,
                            start=True,
                            stop=first,
                            skip_group_check=True,
                        )
                        ps_by_slot = (ps_r, ps_r, ps_z, ps_z, ps_n, ps_n)

                        def _mm(slot, wsb, k, mov, stop):
                            out = ps_by_slot[slot][:, slot % 2]
                            nc.tensor.matmul(
                                out,
                                wsb[:, k, slot],
                                mov[:, k],
                                start=False,
                                stop=stop,
                                skip_group_check=True,
                            )

                        if not first:
                            # h(t-1) = hz_prev - nzm_prev, fed as two moving
                            # operands; nzm pass uses negated weights.  hz
                            # pass executes early (hz ready right after
                            # sigma_z); nzm pass is the critical tail, r
                            # slots first so sigma_r fires after 4 MMs.
                            for m in range(M3):
                                for k in range(KH):
                                    _mm(m, whh_sb, k, hz_prev, False)
                            if WARM:
                                # dummy wide matmuls into a scratch bank:
                                # raise PE-array activity so the HAM clock
                                # gate stays at full rate; result never read.
                                # They fill the PE-idle window while waiting
                                # for nzm, off the critical path.
                                for _ in range(2):
                                    ps_w = pswp.tile(
                                        [128, 4, 128], f32, tag="warm"
                                    )
                                    nc.tensor.matmul(
                                        ps_w,
                                        idw_sb,
                                        whh_sb[:, 0, 0:4],
                                        start=True,
                                        stop=True,
                                        skip_group_check=True,
                                    )
                            for m in (0, 1, 2, 3, 4, 5):
                                for k in range(KH):
                                    _mm(m, whhN_sb, k, nzm_prev, k == KH - 1)
                        rz = spool.tile([128, 4, BL], f32, tag="rz")
                        nc.scalar.activation(rz[:, 0:2], ps_r, AF.Sigmoid)
                        nc.scalar.activation(rz[:, 2:4], ps_z, AF.Sigmoid)
                        if DEBUG_HS and t == 1:
                            nc.sync.dma_start(
                                out=rz_dbg[:, :],
                                in_=rz.rearrange("p m b -> p (m b)"),
                            )
                        # hz = z * h(t-1) — off critical path, feeds next MMs
                        hz_t = hpool.tile([128, KH, BL], wdt, tag="hz")
                        if not first:
                            nc.gpsimd.tensor_mul(
                                hz_t, rz[:, 2:4], hs_sb[:, :, :, t - 1]
                            )
                        # n = tanh(gx_n + r*(gh_n + b_hn));  b_hn already in psum
                        rn = spool.tile([128, KH, BL], f32, tag="rn")
                        nc.vector.tensor_mul(rn, ps_n, rz[:, 0:2])
                        npre = spool.tile([128, KH, BL], f32, tag="npre")
                        nc.vector.tensor_add(
                            npre,
                            rn,
                            gx[:, 4:6, :, :, tcx].rearrange("p m q b -> p m (q b)"),
                        )
                        n_sb = spool.tile([128, KH, BL], f32, tag="n_sb")
                        nc.scalar.activation(n_sb, npre, AF.Tanh)
                        # nzm = (z-1)*n  (so h = hz - nzm)
                        nzm_t = hpool.tile([128, KH, BL], wdt, tag="nzm")
                        nc.vector.scalar_tensor_tensor(
                            nzm_t,
                            rz[:, 2:4],
                            1.0,
                            n_sb,
                            op0=ALU.subtract,
                            op1=ALU.mult,
                        )
                        # materialize h into history (off critical path)
                        if first:
                            nc.gpsimd.tensor_scalar_mul(
                                hs_sb[:, :, :, t], nzm_t, -1.0
                            )
                        else:
                            nc.gpsimd.tensor_sub(
                                hs_sb[:, :, :, t], hz_t, nzm_t
                            )
                        if first:
                            # t=0: h = -nzm only; next step's hz pass runs
                            # against an explicit zero tile
                            nc.gpsimd.memset(hz_t, 0.0)
                        hz_prev = hz_t
                        nzm_prev = nzm_t

            if DEBUG_HS:
                nc.sync.dma_start(
                    out=hs_dbg[:, :],
                    in_=hs_sb.rearrange("p k b t -> p (k b t)"),
                )
            # ---------------- attention ----------------
            with (
                tc.tile_pool(name="att", bufs=1) as apool,
                tc.tile_pool(name="scr2", bufs=2) as s2pool,
                tc.tile_pool(name="psa", bufs=2, space="PSUM") as psap,
                tc.tile_pool(name="psb", bufs=3, space="PSUM") as psbp,
                tc.tile_pool(name="pss", bufs=1, space="PSUM") as pssp,
            ):
                BT = BL * T_
                nj = BT // 512
                # um = tanh(wv_W @ hs + wv_b): [A, (b,t)] fp16
                um_sb = apool.tile([A, BL, T_], f16)
                umf = um_sb.rearrange("a b t -> a (b t)")
                hsf = hs_sb.rearrange("p k b t -> p k (b t)")
                for j in range(nj):
                    ps_um = psap.tile([A, 512], f32, tag="ps_um")
                    for k in range(KH):
                        nc.tensor.matmul(
                            ps_um,
                            wv_sb[:, k],
                            hsf[:, k, j * 512 : (j + 1) * 512],
                            start=(k == 0),
                            stop=(k == KH - 1),
                        )
                    nc.scalar.activation(
                        umf[:, j * 512 : (j + 1) * 512], ps_um, AF.Tanh, bias=wvb_sb
                    )
                # s[b, t] = sum_a wu[a] um[a, b, t]  -> psum [BL, T]
                ps_s = pssp.tile([BL, T_], f32)
                nth = T_ // 512 if T_ >= 512 else 1
                tsz = min(512, T_)
                for th in range(nth):
                    for b in range(BL):
                        nc.tensor.matmul(
                            ps_s[:, th * tsz : (th + 1) * tsz],
                            wud_sb[:, b],
                            um_sb[:, b, th * tsz : (th + 1) * tsz],
                            start=(b == 0),
                            stop=(b == BL - 1),
                        )
                # softmax over t (free dim)
                nm = s2pool.tile([BL, 1], f32)
                nc.vector.reduce_max(nm, ps_s, axis=AX.X, negate=True)
                expw = s2pool.tile([BL, T_], f32)
                se = s2pool.tile([BL, 1], f32)
                nc.scalar.activation(expw, ps_s, AF.Exp, bias=nm, accum_out=se)
                rse = s2pool.tile([BL, 1], f32)
                nc.vector.reciprocal(rse, se)
                alpha = s2pool.tile([BL, T_], f16)
                nc.vector.tensor_scalar_mul(alpha, expw, rse)
                # context: ctxT[p, k, b] = sum_t hs[p,k,b,t] * alpha[b,t].
                # alpha row b broadcast to 128 partitions via sel matmul; the
                # fused multiply+reduce reads the psum directly.
                ctxT = apool.tile([128, KH, BL], f32)
                for b in range(BL):
                    parts = [[None] * nth for _ in range(KH)]
                    for th in range(nth):
                        ps_ab = psbp.tile([128, tsz], f32, tag="ps_ab")
                        nc.tensor.matmul(
                            ps_ab,
                            sel_sb[:, b],
                            alpha[:, th * tsz : (th + 1) * tsz],
                            start=True,
                            stop=True,
                        )
                        if USE_TTR:
                            for k in range(KH):
                                scr = s2pool.tile([128, tsz], f32, tag="ttr_scr")
                                part = s2pool.tile(
                                    [128, 1], f32, tag="ttr_acc", bufs=8
                                )
                                nc.vector.tensor_tensor_reduce(
                                    out=scr,
                                    in0=hs_sb[:, k, b, th * tsz : (th + 1) * tsz],
                                    in1=ps_ab,
                                    scale=1.0,
                                    scalar=0.0,
                                    op0=ALU.mult,
                                    op1=ALU.add,
                                    accum_out=part,
                                )
                                parts[k][th] = part
                        else:
                            ab = s2pool.tile([128, tsz], f16, tag="ab_sb")
                            ceng = nc.vector if b % 2 == 0 else nc.scalar
                            if ceng is nc.vector:
                                nc.vector.tensor_copy(out=ab, in_=ps_ab)
                            else:
                                nc.scalar.copy(out=ab, in_=ps_ab)
                            for k in range(KH):
                                # fused multiply + accumulate-reduce in one
                                # DVE op via stt's accum_out
                                scr = s2pool.tile([128, tsz], f16, tag="ttr_scr")
                                part = s2pool.tile(
                                    [128, 1], f32, tag="ttr_acc", bufs=8
                                )
                                nc.vector.scalar_tensor_tensor(
                                    scr,
                                    hs_sb[:, k, b, th * tsz : (th + 1) * tsz],
                                    1.0,
                                    ab,
                                    op0=ALU.mult,
                                    op1=ALU.mult,
                                    accum_out=part,
                                )
                                parts[k][th] = part
                    for k in range(KH):
                        if nth == 1:
                            nc.vector.tensor_copy(
                                out=ctxT[:, k, b : b + 1], in_=parts[k][0]
                            )
                        else:
                            nc.vector.tensor_add(
                                ctxT[:, k, b : b + 1], parts[k][0], parts[k][1]
                            )
                # out = h2o_W . ctx + h2o_b
                ps_o = pssp.tile([1, BL], f32, tag="ps_o")
                for k in range(KH):
                    nc.tensor.matmul(
                        ps_o,
                        h2o_sb[:, k : k + 1],
                        ctxT[:, k],
                        start=(k == 0),
                        stop=(k == KH - 1),
                    )
                o_sb = s2pool.tile([1, BL], f32)
                nc.vector.tensor_scalar_add(o_sb, ps_o, h2ob_sb)
                nc.sync.dma_start(
                    out=out_ext[:, :].rearrange("b one -> one b"), in_=o_sb
                )
    nc.compile()
    return nc


def _prep_maps(inputs, T_):
    x = np.ascontiguousarray(np.asarray(inputs["x"], dtype=np.float32)[:, :T_, :])
    W_ih = np.asarray(inputs["W_ih"], dtype=np.float32)
    W_hh = np.asarray(inputs["W_hh"], dtype=np.float32)
    b_ih = np.asarray(inputs["b_ih"], dtype=np.float32)
    b_hh = np.asarray(inputs["b_hh"], dtype=np.float32)
    wv_W = np.asarray(inputs["wv_W"], dtype=np.float32)
    wv_b = np.asarray(inputs["wv_b"], dtype=np.float32)
    wu = np.asarray(inputs["wu"], dtype=np.float32)
    h2o_W = np.asarray(inputs["h2o_W"], dtype=np.float32)
    h2o_b = np.asarray(inputs["h2o_b"], dtype=np.float32)

    w_np = np.float16 if WDT_F16 else np.float32
    whhT = np.ascontiguousarray(W_hh.T).astype(w_np)  # [H, 3H]
    whhTN = np.ascontiguousarray(-W_hh.T).astype(w_np)  # [H, 3H] negated
    bhn = b_hh[512:768].reshape(2, 128).T  # [128, KH]
    bhn_pack = np.broadcast_to(
        bhn[:, :, None], (128, 2, BL)
    ).reshape(128, 2 * BL).astype(w_np)
    wihT = np.ascontiguousarray(W_ih.T).astype(w_np)  # [I, 3H]
    bsum = b_ih + b_hh
    bias_pack = np.stack(
        [bsum[0:128], bsum[128:256], bsum[256:384], bsum[384:512],
         b_ih[512:640], b_ih[640:768], b_hh[512:640], b_hh[640:768]],
        axis=1,
    ).astype(np.float32)  # [128, 8]
    identity = np.eye(128, dtype=np.float32)
    wvT = np.ascontiguousarray(wv_W.T).astype(np.float16)  # [H, A]
    wvb_ = wv_b.reshape(A, 1).astype(np.float32)
    wud = (wu[:, None, None] * np.eye(BL, dtype=np.float32)[None]).reshape(
        A, BL * BL
    ).astype(np.float16)
    sel = np.repeat(np.eye(BL, dtype=np.float32), 128, axis=1).astype(np.float16)
    h2o_pack = np.ascontiguousarray(
        h2o_W.reshape(KH, 128).T
    ).astype(np.float32)  # [128, KH]
    h2ob_ = h2o_b.reshape(1, 1).astype(np.float32)

    shared = dict(
        W_hhT=whhT, W_hhTN=whhTN, bhn_pack=bhn_pack,
        W_ihT=wihT, bias_pack=bias_pack, identity=identity,
        wv_WT=wvT, wv_b=wvb_, wu_delta=wud, bcast_sel=sel,
        h2o_pack=h2o_pack, h2o_b=h2ob_,
    )
    maps = []
    for c in range(NCORES):
        m = dict(shared)
        m["x"] = np.ascontiguousarray(x[c * BL : (c + 1) * BL])
        maps.append(m)
    return maps


def _execute(inputs, T_=None, trace=False, tmpdir=None, nc=None):
    T_ = T_ or int(os.environ.get("RNN_T", T))
    if nc is None:
        nc = build_program(T_=T_)
    maps = _prep_maps(inputs, T_)
    res = run_bass_kernel_spmd(
        nc, maps, list(range(NCORES)), trace=trace, tmpdir=tmpdir
    )
    out = np.concatenate([res.results[c]["out"] for c in range(NCORES)], axis=0)
    return out.astype(np.float32), res


def kernel(**inputs):
    out, _ = _execute(inputs)
    return out



# revision 6
# speedup vs baseline: 1.0948x; 1.0026x over previous
"""Att_RNN_GRU Trainium2 Bass kernel, latency-optimized recurrence.

Data-parallel over batch: B=128 split across 8 cores (16 each).
Layout strategy: channels-on-partitions, (batch,time)-on-free everywhere.

Chain-minimized GRU step (critical path after the psum matmuls):
  sigma(ps[0:4]) -> rn = ps_n * r -> npre = rn + gx_n -> tanh -> nzm=(z-1)*n
h is kept as the UNMATERIALIZED pair (hz, -nzm): the next step's matmuls
accumulate W*hz (via whh) and W*nzm (via negated weights whhN), so the
h materialization (hs history write, m = hz - nzm) happens off the
critical path on gpsimd.  b_hn enters psum via a constant-tile identity
matmul (inject2); the r,z,n gx enter via inject1 (identity matmul on the
precomputed gx chunk, biases baked).
Attention phase unchanged from baseline (fp16 um/softmax/context).
"""

import os

import numpy as np

import concourse.bass as bass
import concourse.mybir as mybir
from concourse import bacc
from concourse import bass_utils as _bu
from concourse.bass_utils import run_bass_kernel_spmd

_orig_run_command = _bu.run_command


def _run_command_nobs(cmd, **kw):
    cmd = [
        ("--enable-birsim=false" if c == "--enable-birsim=true" else c) for c in cmd
    ]
    return _orig_run_command(cmd, **kw)


_bu.run_command = _run_command_nobs
from concourse.tile import TileContext

B, T, I, H, A = 128, 1024, 128, 256, 40
NCORES = 8
BL = B // NCORES  # 16 batch rows per core
KH = H // 128  # 2 hidden k-chunks
M3 = 3 * H // 128  # 6 gate m-chunks (r: 0,1  z: 2,3  n: 4,5)
CH = 64  # timesteps per gx chunk

f32 = mybir.dt.float32
f16 = mybir.dt.float16
WDT_F16 = os.environ.get("RNN_WDT", "f16") == "f16"
TWOSUM = os.environ.get("RNN_TWOSUM", "1") == "1"
DEBUG_HS = os.environ.get("RNN_DEBUG_HS", "0") == "1"
USE_TTR = os.environ.get("RNN_TTR", "0") == "1"
WARM = os.environ.get("RNN_WARM", "0") == "1"
wdt = f16 if WDT_F16 else f32

AF = mybir.ActivationFunctionType
ALU = mybir.AluOpType
AX = mybir.AxisListType


def build_program(T_=None, CH_=None):
    T_ = T_ or int(os.environ.get("RNN_T", T))
    CH_ = CH_ or int(os.environ.get("RNN_CH", CH))
    nchunk = T_ // CH_
    assert T_ % CH_ == 0

    nc = bacc.Bacc(
        "TRN2", target_bir_lowering=False, debug=False, num_devices=NCORES
    )
    x_ext = nc.declare_dram_parameter("x", [BL, T_, I], f32, isOutput=False)
    whhT = nc.declare_dram_parameter("W_hhT", [H, 3 * H], wdt, isOutput=False)
    whhTN = nc.declare_dram_parameter("W_hhTN", [H, 3 * H], wdt, isOutput=False)
    bhnp = nc.declare_dram_parameter("bhn_pack", [128, 2 * BL], wdt, isOutput=False)
    wihT = nc.declare_dram_parameter("W_ihT", [I, 3 * H], wdt, isOutput=False)
    biasp = nc.declare_dram_parameter("bias_pack", [128, 8], f32, isOutput=False)
    ident = nc.declare_dram_parameter("identity", [128, 128], f32, isOutput=False)
    wvT = nc.declare_dram_parameter("wv_WT", [H, A], f16, isOutput=False)
    wvb = nc.declare_dram_parameter("wv_b", [A, 1], f32, isOutput=False)
    wud = nc.declare_dram_parameter("wu_delta", [A, BL * BL], f16, isOutput=False)
    sel = nc.declare_dram_parameter("bcast_sel", [BL, BL * 128], f16, isOutput=False)
    h2o = nc.declare_dram_parameter("h2o_pack", [128, KH], f32, isOutput=False)
    h2ob = nc.declare_dram_parameter("h2o_b", [1, 1], f32, isOutput=False)
    out_ext = nc.declare_dram_parameter("out", [BL, 1], f32, isOutput=True)
    hs_dbg = (
        nc.declare_dram_parameter("hs_dbg", [128, KH * BL * T_], f16, isOutput=True)
        if DEBUG_HS
        else None
    )
    rz_dbg = (
        nc.declare_dram_parameter("rz_dbg", [128, 4 * BL], f32, isOutput=True)
        if DEBUG_HS
        else None
    )

    with TileContext(nc) as tc:
        with (
            tc.tile_pool(name="consts", bufs=1) as cpool,
            tc.tile_pool(name="hspool", bufs=1) as hspool,
        ):
            # ---------------- constants ----------------
            whh_sb = cpool.tile([128, KH, M3, 128], wdt)
            for k in range(KH):
                nc.sync.dma_start(
                    out=whh_sb[:, k],
                    in_=whhT[k * 128 : (k + 1) * 128, :].rearrange(
                        "p (m c) -> p m c", m=M3
                    ),
                )
            # consts split across both HWDGE queues (SP + ACT) so the
            # startup DMA train doesn't serialize on one ring
            whhN_sb = cpool.tile([128, KH, M3, 128], wdt)
            for k in range(KH):
                nc.scalar.dma_start(
                    out=whhN_sb[:, k],
                    in_=whhTN[k * 128 : (k + 1) * 128, :].rearrange(
                        "p (m c) -> p m c", m=M3
                    ),
                )
            bhn_sb = cpool.tile([128, KH, BL], wdt)
            nc.scalar.dma_start(
                out=bhn_sb, in_=bhnp[:, :].rearrange("p (k b) -> p k b", k=KH)
            )
            wih_sb = cpool.tile([128, M3, 128], wdt)
            nc.sync.dma_start(
                out=wih_sb, in_=wihT[:, :].rearrange("p (m c) -> p m c", m=M3)
            )
            bias_sb = cpool.tile([128, 8], f32)
            nc.sync.dma_start(out=bias_sb, in_=biasp[:, :])
            id_sb = cpool.tile([128, 128], f32)
            nc.sync.dma_start(out=id_sb, in_=ident[:, :])
            idw_sb = cpool.tile([128, 128], wdt)
            if WDT_F16:
                nc.vector.tensor_copy(out=idw_sb, in_=id_sb)
            else:
                idw_sb = id_sb
            wv_sb = cpool.tile([128, KH, A], f16)
            for k in range(KH):
                nc.scalar.dma_start(
                    out=wv_sb[:, k], in_=wvT[k * 128 : (k + 1) * 128, :]
                )
            wvb_sb = cpool.tile([A, 1], f32)
            nc.scalar.dma_start(out=wvb_sb, in_=wvb[:, :])
            wud_sb = cpool.tile([A, BL, BL], f16)
            nc.scalar.dma_start(
                out=wud_sb, in_=wud[:, :].rearrange("a (b c) -> a b c", b=BL)
            )
            sel_sb = cpool.tile([BL, BL, 128], f16)
            nc.scalar.dma_start(
                out=sel_sb, in_=sel[:, :].rearrange("a (b c) -> a b c", b=BL)
            )
            h2o_sb = cpool.tile([128, KH], f32)
            nc.scalar.dma_start(out=h2o_sb, in_=h2o[:, :])
            h2ob_sb = cpool.tile([1, 1], f32)
            nc.scalar.dma_start(out=h2ob_sb, in_=h2ob[:, :])

            # hidden-state history, fp16: [128, k, b, t]
            hs_sb = hspool.tile([128, KH, BL, T_], f16)

            # ---------------- recurrence ----------------
            with (
                tc.tile_pool(name="xio", bufs=2) as xpool,
                tc.tile_pool(name="gxp", bufs=2) as gxpool,
                tc.tile_pool(name="state", bufs=6) as hpool,
                tc.tile_pool(name="scr", bufs=4) as spool,
                tc.tile_pool(name="psr", bufs=1, space="PSUM") as psrp,
                tc.tile_pool(name="psz", bufs=1, space="PSUM") as pszp,
                tc.tile_pool(name="psgn", bufs=2, space="PSUM") as psgn,
                tc.tile_pool(name="pst", bufs=2, space="PSUM") as pstp,
                tc.tile_pool(name="psx", bufs=2, space="PSUM") as psxp,
                tc.tile_pool(name="psw", bufs=1, space="PSUM") as pswp,
            ):
                hz_prev = None
                nzm_prev = None

                for c in range(nchunk):
                    t0 = c * CH_
                    npair = BL // 2
                    ncol = BL * CH_  # gx cols this chunk
                    # ---- load x chunk, pair-major rows: [(bo t), i]
                    xin = xpool.tile([128, npair * 128], f32, tag="xin")
                    xiv = xin.rearrange("p (q c) -> p q c", q=npair)
                    for p in range(npair):
                        for bo in range(2):
                            nc.sync.dma_start(
                                out=xiv[bo * CH_ : (bo + 1) * CH_, p],
                                in_=x_ext[2 * p + bo, t0 : t0 + CH_, :],
                            )
                    # ---- transpose -> xT [i, (pair, bo, tc)]
                    xT = xpool.tile([128, ncol], wdt, tag="xT")
                    for p in range(npair):
                        ps_t = pstp.tile([128, 128], f32, tag="ps_t")
                        nc.tensor.transpose(ps_t, xiv[:, p], id_sb)
                        nc.scalar.copy(out=xT[:, p * 128 : (p + 1) * 128], in_=ps_t)
                    # ---- gx matmuls: [128, m, pair, bo, tc] with bias baked
                    # slots 0-3: r,z (bias = b_ih+b_hh); slots 4,5: n (bias = b_ih)
                    gx = gxpool.tile([128, M3, npair, 2, CH_], wdt, tag="gx")
                    gxf = gx.rearrange("p m q b t -> p m (q b t)")
                    nh = ncol // 512 if ncol >= 512 else 1
                    nsz = min(512, ncol)
                    for m in range(M3):
                        for j in range(nh):
                            ps_gx = psxp.tile([128, nsz], f32, tag="ps_gx")
                            nc.tensor.matmul(
                                ps_gx,
                                wih_sb[:, m],
                                xT[:, j * nsz : (j + 1) * nsz],
                                start=True,
                                stop=True,
                            )
                            if (m + j) % 2 == 0:
                                nc.vector.tensor_scalar_add(
                                    gxf[:, m, j * nsz : (j + 1) * nsz],
                                    ps_gx,
                                    bias_sb[:, m : m + 1],
                                )
                            else:
                                nc.scalar.activation(
                                    gxf[:, m, j * nsz : (j + 1) * nsz],
                                    ps_gx,
                                    AF.Identity,
                                    bias=bias_sb[:, m : m + 1],
                                )
                    # ---- CH_ recurrence steps
                    for tcx in range(CH_):
                        t = t0 + tcx
                        first = t == 0
                        # three psum tiles (= separate banks/semaphores):
                        # sigma_r fires after only the r-slot writers; z and
                        # n slots finish later, off the critical head.
                        ps_r = psrp.tile([128, 2, BL], f32, tag="ps_r")
                        ps_z = pszp.tile([128, 2, BL], f32, tag="ps_z")
                        ps_n = psgn.tile([128, KH, BL], f32, tag="ps_n")
                        nc.tensor.matmul(
                            ps_r,
                            idw_sb,
                            gx[:, 0:2, :, :, tcx],
                            start=True,
                            stop=first,
                            skip_group_check=True,
                        )
                        nc.tensor.matmul(
                            ps_z,
                            idw_sb,
                            gx[:, 2:4, :, :, tcx],
                            start=True,
                            stop=first,
                            skip_group_check=True,
                        )
                        nc.tensor.matmul(
                            ps_n,
                            idw_sb,
                            bhn_sb,
                            start=True,
                            stop=first,
                            skip_group_check=True,
                        )
                        ps_by_slot = (ps_r, ps_r, ps_z, ps_z, ps_n, ps_n)

                        def _mm(slot, wsb, k, mov, stop):
                            out = ps_by_slot[slot][:, slot % 2]
                            nc.tensor.matmul(
                                out,
                                wsb[:, k, slot],
                                mov[:, k],
                                start=False,
                                stop=stop,
                                skip_group_check=True,
                            )

                        if not first:
                            # h(t-1) = hz_prev - nzm_prev, fed as two moving
                            # operands; nzm pass uses negated weights.  hz
                            # pass executes early (hz ready right after
                            # sigma_z); nzm pass is the critical tail, r
                            # slots first so sigma_r fires after 4 MMs.
                            for m in range(M3):
                                for k in range(KH):
                                    _mm(m, whh_sb, k, hz_prev, False)
                            if WARM:
                                # dummy wide matmuls into a scratch bank:
                                # raise PE-array activity so the HAM clock
                                # gate stays at full rate; result never read.
                                # They fill the PE-idle window while waiting
                                # for nzm, off the critical path.
                                for _ in range(2):
                                    ps_w = pswp.tile(
                                        [128, 4, 128], f32, tag="warm"
                                    )
                                    nc.tensor.matmul(
                                        ps_w,
                                        idw_sb,
                                        whh_sb[:, 0, 0:4],
                                        start=True,
                                        stop=True,
                                        skip_group_check=True,
                                    )
                            for m in (0, 1, 2, 3, 4, 5):
                                for k in range(KH):
                                    _mm(m, whhN_sb, k, nzm_prev, k == KH - 1)
                        rz = spool.tile([128, 4, BL], f32, tag="rz")
                        nc.scalar.activation(rz[:, 0:2], ps_r, AF.Sigmoid)
                        nc.scalar.activation(rz[:, 2:4], ps_z, AF.Sigmoid)
                        if DEBUG_HS and t == 1:
                            nc.sync.dma_start(
                                out=rz_dbg[:, :],
                                in_=rz.rearrange("p m b -> p (m b)"),
                            )
                        # hz = z * h(t-1) — off critical path, feeds next MMs
                        hz_t = hpool.tile([128, KH, BL], wdt, tag="hz")
                        if not first:
                            nc.gpsimd.tensor_mul(
                                hz_t, rz[:, 2:4], hs_sb[:, :, :, t - 1]
                            )
                        # n = tanh(gx_n + r*(gh_n + b_hn));  b_hn already in psum
                        rn = spool.tile([128, KH, BL], f32, tag="rn")
                        nc.vector.tensor_mul(rn, ps_n, rz[:, 0:2])
                        npre = spool.tile([128, KH, BL], f32, tag="npre")
                        nc.vector.tensor_add(
                            npre,
                            rn,
                            gx[:, 4:6, :, :, tcx].rearrange("p m q b -> p m (q b)"),
                        )
                        n_sb = spool.tile([128, KH, BL], f32, tag="n_sb")
                        nc.scalar.activation(n_sb, npre, AF.Tanh)
                        # nzm = (z-1)*n  (so h = hz - nzm)
                        nzm_t = hpool.tile([128, KH, BL], wdt, tag="nzm")
                        nc.vector.scalar_tensor_tensor(
                            nzm_t,
                            rz[:, 2:4],
                            1.0,
                            n_sb,
                            op0=ALU.subtract,
                            op1=ALU.mult,
                        )
                        # materialize h into history (off critical path)
                        if first:
                            nc.gpsimd.tensor_scalar_mul(
                                hs_sb[:, :, :, t], nzm_t, -1.0
                            )
                        else:
                            nc.gpsimd.tensor_sub(
                                hs_sb[:, :, :, t], hz_t, nzm_t
                            )
                        if first:
                            # t=0: h = -nzm only; next step's hz pass runs
                            # against an explicit zero tile
                            nc.gpsimd.memset(hz_t, 0.0)
                        hz_prev = hz_t
                        nzm_prev = nzm_t

            if DEBUG_HS:
                nc.sync.dma_start(
                    out=hs_dbg[:, :],
                    in_=hs_sb.rearrange("p k b t -> p (k b t)"),
                )
            # ---------------- attention ----------------
            with (
                tc.tile_pool(name="att", bufs=1) as apool,
                tc.tile_pool(name="scr2", bufs=2) as s2pool,
                tc.tile_pool(name="psa", bufs=2, space="PSUM") as psap,
                tc.tile_pool(name="psb", bufs=3, space="PSUM") as psbp,
                tc.tile_pool(name="pss", bufs=1, space="PSUM") as pssp,
            ):
                BT = BL * T_
                nj = BT // 512
                # um = tanh(wv_W @ hs + wv_b): [A, (b,t)] fp16
                um_sb = apool.tile([A, BL, T_], f16)
                umf = um_sb.rearrange("a b t -> a (b t)")
                hsf = hs_sb.rearrange("p k b t -> p k (b t)")
                for j in range(nj):
                    ps_um = psap.tile([A, 512], f32, tag="ps_um")
                    for k in range(KH):
                        nc.tensor.matmul(
                            ps_um,
                            wv_sb[:, k],
                            hsf[:, k, j * 512 : (j + 1) * 512],
                            start=(k == 0),
                            stop=(k == KH - 1),
                        )
                    nc.scalar.activation(
                        umf[:, j * 512 : (j + 1) * 512], ps_um, AF.Tanh, bias=wvb_sb
                    )
                # s[b, t] = sum_a wu[a] um[a, b, t]  -> psum [BL, T]
                ps_s = pssp.tile([BL, T_], f32)
                nth = T_ // 512 if T_ >= 512 else 1
                tsz = min(512, T_)
                for th in range(nth):
                    for b in range(BL):
                        nc.tensor.matmul(
                            ps_s[:, th * tsz : (th + 1) * tsz],
                            wud_sb[:, b],
                            um_sb[:, b, th * tsz : (th + 1) * tsz],
                            start=(b == 0),
                            stop=(b == BL - 1),
                        )
                # softmax over t (free dim)
                nm = s2pool.tile([BL, 1], f32)
                nc.vector.reduce_max(nm, ps_s, axis=AX.X, negate=True)
                expw = s2pool.tile([BL, T_], f32)
                se = s2pool.tile([BL, 1], f32)
                nc.scalar.activation(expw, ps_s, AF.Exp, bias=nm, accum_out=se)
                rse = s2pool.tile([BL, 1], f32)
                nc.vector.reciprocal(rse, se)
                alpha = s2pool.tile([BL, T_], f16)
                nc.vector.tensor_scalar_mul(alpha, expw, rse)
                # context: ctxT[p, k, b] = sum_t hs[p,k,b,t] * alpha[b,t].
                # alpha row b broadcast to 128 partitions via sel matmul; the
                # fused multiply+reduce reads the psum directly.
                ctxT = apool.tile([128, KH, BL], f32)
                for b in range(BL):
                    parts = [[None] * nth for _ in range(KH)]
                    for th in range(nth):
                        ps_ab = psbp.tile([128, tsz], f32, tag="ps_ab")
                        nc.tensor.matmul(
                            ps_ab,
                            sel_sb[:, b],
                            alpha[:, th * tsz : (th + 1) * tsz],
                            start=True,
                            stop=True,
                        )
                        if USE_TTR:
                            for k in range(KH):
                                scr = s2pool.tile([128, tsz], f32, tag="ttr_scr")
                                part = s2pool.tile(
                                    [128, 1], f32, tag="ttr_acc", bufs=8
                                )
                                nc.vector.tensor_tensor_reduce(
                                    out=scr,
                                    in0=hs_sb[:, k, b, th * tsz : (th + 1) * tsz],
                                    in1=ps_ab,
                                    scale=1.0,
                                    scalar=0.0,
                                    op0=ALU.mult,
                                    op1=ALU.add,
                                    accum_out=part,
                                )
                                parts[k][th] = part
                        else:
                            ab = s2pool.tile([128, tsz], f16, tag="ab_sb")
                            ceng = nc.vector if b % 2 == 0 else nc.scalar
                            if ceng is nc.vector:
                                nc.vector.tensor_copy(out=ab, in_=ps_ab)
                            else:
                                nc.scalar.copy(out=ab, in_=ps_ab)
                            for k in range(KH):
                                # fused multiply + accumulate-reduce in one
                                # DVE op via stt's accum_out
                                scr = s2pool.tile([128, tsz], f16, tag="ttr_scr")
                                part = s2pool.tile(
                                    [128, 1], f32, tag="ttr_acc", bufs=8
                                )
                                nc.vector.scalar_tensor_tensor(
                                    scr,
                                    hs_sb[:, k, b, th * tsz : (th + 1) * tsz],
                                    1.0,
                                    ab,
                                    op0=ALU.mult,
                                    op1=ALU.mult,
                                    accum_out=part,
                                )
                                parts[k][th] = part
                    for k in range(KH):
                        if nth == 1:
                            nc.vector.tensor_copy(
                                out=ctxT[:, k, b : b + 1], in_=parts[k][0]
                            )
                        else:
                            nc.vector.tensor_add(
                                ctxT[:, k, b : b + 1], parts[k][0], parts[k][1]
                            )
                # out = h2o_W . ctx + h2o_b
                ps_o = pssp.tile([1, BL], f32, tag="ps_o")
                for k in range(KH):
                    nc.tensor.matmul(
                        ps_o,
                        h2o_sb[:, k : k + 1],
                        ctxT[:, k],
                        start=(k == 0),
                        stop=(k == KH - 1),
                    )
                o_sb = s2pool.tile([1, BL], f32)
                nc.vector.tensor_scalar_add(o_sb, ps_o, h2ob_sb)
                nc.sync.dma_start(
                    out=out_ext[:, :].rearrange("b one -> one b"), in_=o_sb
                )
    nc.compile()
    return nc


def _prep_maps(inputs, T_):
    x = np.ascontiguousarray(np.asarray(inputs["x"], dtype=np.float32)[:, :T_, :])
    W_ih = np.asarray(inputs["W_ih"], dtype=np.float32)
    W_hh = np.asarray(inputs["W_hh"], dtype=np.float32)
    b_ih = np.asarray(inputs["b_ih"], dtype=np.float32)
    b_hh = np.asarray(inputs["b_hh"], dtype=np.float32)
    wv_W = np.asarray(inputs["wv_W"], dtype=np.float32)
    wv_b = np.asarray(inputs["wv_b"], dtype=np.float32)
    wu = np.asarray(inputs["wu"], dtype=np.float32)
    h2o_W = np.asarray(inputs["h2o_W"], dtype=np.float32)
    h2o_b = np.asarray(inputs["h2o_b"], dtype=np.float32)

    w_np = np.float16 if WDT_F16 else np.float32
    whhT = np.ascontiguousarray(W_hh.T).astype(w_np)  # [H, 3H]
    whhTN = np.ascontiguousarray(-W_hh.T).astype(w_np)  # [H, 3H] negated
    bhn = b_hh[512:768].reshape(2, 128).T  # [128, KH]
    bhn_pack = np.broadcast_to(
        bhn[:, :, None], (128, 2, BL)
    ).reshape(128, 2 * BL).astype(w_np)
    wihT = np.ascontiguousarray(W_ih.T).astype(w_np)  # [I, 3H]
    bsum = b_ih + b_hh
    bias_pack = np.stack(
        [bsum[0:128], bsum[128:256], bsum[256:384], bsum[384:512],
         b_ih[512:640], b_ih[640:768], b_hh[512:640], b_hh[640:768]],
        axis=1,
    ).astype(np.float32)  # [128, 8]
    identity = np.eye(128, dtype=np.float32)
    wvT = np.ascontiguousarray(wv_W.T).astype(np.float16)  # [H, A]
    wvb_ = wv_b.reshape(A, 1).astype(np.float32)
    wud = (wu[:, None, None] * np.eye(BL, dtype=np.float32)[None]).reshape(
        A, BL * BL
    ).astype(np.float16)
    sel = np.repeat(np.eye(BL, dtype=np.float32), 128, axis=1).astype(np.float16)
    h2o_pack = np.ascontiguousarray(
        h2o_W.reshape(KH, 128).T
    ).astype(np.float32)  # [128, KH]
    h2ob_ = h2o_b.reshape(1, 1).astype(np.float32)

    shared = dict(
        W_hhT=whhT, W_hhTN=whhTN, bhn_pack=bhn_pack,
        W_ihT=wihT, bias_pack=bias_pack, identity=identity,
        wv_WT=wvT, wv_b=wvb_, wu_delta=wud, bcast_sel=sel,
        h2o_pack=h2o_pack, h2o_b=h2ob_,
    )
    maps = []
    for c in range(NCORES):
        m = dict(shared)
        m["x"] = np.ascontiguousarray(x[c * BL : (c + 1) * BL])
        maps.append(m)
    return maps


def _execute(inputs, T_=None, trace=False, tmpdir=None, nc=None):
    T_ = T_ or int(os.environ.get("RNN_T", T))
    if nc is None:
        nc = build_program(T_=T_)
    maps = _prep_maps(inputs, T_)
    res = run_bass_kernel_spmd(
        nc, maps, list(range(NCORES)), trace=trace, tmpdir=tmpdir
    )
    out = np.concatenate([res.results[c]["out"] for c in range(NCORES)], axis=0)
    return out.astype(np.float32), res


def kernel(**inputs):
    out, _ = _execute(inputs)
    return out

